# revision 11
# baseline (speedup 1.0000x reference)
"""AttentionBlock (GroupNorm + single-head spatial attention + SE gate + residual)
Trainium2 Bass/Tile kernel, data-parallel over batch across 8 NeuronCores.

Full shapes: x [32, 256, 32, 32] f32 -> out [32, 256, 32, 32] f32.
Per core: 4 samples. Per sample (C=256, N=1024):
  xn = GroupNorm(x) (32 groups)            [C, N]  (bf16)
  q, k = Wqk @ xn                          [2C, N] (bf16, [c,n] layout)
  vT = xn^T @ WvT                          [N, C]  (bf16, [n,c] layout - direct!)
  esT = exp((k^T q) / 16)                  [N, N]  ([j, i] layout, j = softmax axis)
  sums_bc = ones128 @ esT  (accum over j)  [128, N] (each row = sum_j exp)
  r = 1/sums (reciprocal_approx_fast)      [128, N]
  xat = (vT^T @ esT) * r                   [C, N]  (unnormalized AV, scaled after)
  y = Wp @ xat                             [C, N]
  out = x + (y + bp) * gate[c]             (gate = SE sigmoid path from channel means)

No transposes anywhere: softmax reductions over j land on the PE contraction
axis (ones-matmul), the normalization is a rank-1 column scale folded in after
the AV matmul.
"""

import numpy as np
import ml_dtypes

B, C, HW, N = 32, 256, 32, 1024
NCORES = 8
BL = B // NCORES          # samples per core
GROUPS = 32
GSIZE = C // GROUPS       # 8 channels per group
EPS = 1e-5
CT = 2                    # channel partition tiles (256 = 2*128)
P = 128

_CACHE = {}


def _build_program(want_bias_v):
    import concourse.bacc as bacc
    import concourse.mybir as mybir
    import concourse.tile as tile

    f32 = mybir.dt.float32
    bf16 = mybir.dt.bfloat16
    AX = mybir.AxisListType.X
    AF = mybir.ActivationFunctionType
    ALU = mybir.AluOpType

    nc = bacc.Bacc()

    # ---- DRAM I/O ----
    x_d = nc.dram_tensor("x", [BL, C, N], f32, kind="ExternalInput")
    out_d = nc.dram_tensor("out", [BL, C, N], f32, kind="ExternalOutput")
    wqk_d = nc.dram_tensor("wqk", [P, 2, 512], bf16, kind="ExternalInput")
    wv_d = nc.dram_tensor("wv", [P, 2, C], bf16, kind="ExternalInput")
    wp_d = nc.dram_tensor("wp", [P, 2, C], bf16, kind="ExternalInput")
    w1_d = nc.dram_tensor("w1", [P, 2, 64], f32, kind="ExternalInput")
    w2_d = nc.dram_tensor("w2", [64, C], f32, kind="ExternalInput")
    gamma_d = nc.dram_tensor("gamma", [P, 2], f32, kind="ExternalInput")
    beta_d = nc.dram_tensor("beta", [P, 2], f32, kind="ExternalInput")
    bqk_d = nc.dram_tensor("bqk", [P, 4], f32, kind="ExternalInput")
    bv_d = nc.dram_tensor("bv", [P, 2], f32, kind="ExternalInput")
    bp_d = nc.dram_tensor("bp", [P, 2], f32, kind="ExternalInput")
    b1_d = nc.dram_tensor("b1", [64, 1], f32, kind="ExternalInput")
    b2_d = nc.dram_tensor("b2", [P, 2], f32, kind="ExternalInput")
    gm_d = nc.dram_tensor("gm", [P, 16], f32, kind="ExternalInput")
    gmt_d = nc.dram_tensor("gmt", [16, P], f32, kind="ExternalInput")
    ones_d = nc.dram_tensor("ones", [P, P], bf16, kind="ExternalInput")

    with tile.TileContext(nc) as tc:
        with (
            tc.tile_pool(name="persist", bufs=1) as persist,
            tc.tile_pool(name="qk", bufs=2) as qk_pool,
            tc.tile_pool(name="vt", bufs=2) as vt_pool,
            tc.tile_pool(name="es", bufs=2) as es_pool,
            tc.tile_pool(name="xat", bufs=2) as xat_pool,
            tc.tile_pool(name="rr", bufs=2) as r_pool,
            tc.tile_pool(name="junk", bufs=2) as junk_pool,
            tc.tile_pool(name="tree", bufs=2) as tree_pool,
            tc.tile_pool(name="outp", bufs=3) as out_pool,
            tc.tile_pool(name="psb", bufs=3, space="PSUM") as psum_big,
            tc.tile_pool(name="pss", bufs=2, space="PSUM") as psum_small,
        ):
            # ---- DMA prologue: tiny consts, then x slices interleaved
            # with the big weights so sample 0's stats start ASAP (the
            # HWDGE queue is FIFO in emission order).
            gm_sb = persist.tile([P, 16], f32)
            nc.sync.dma_start(out=gm_sb, in_=gm_d[:, :])
            gmt_sb = persist.tile([16, P], f32)
            nc.sync.dma_start(out=gmt_sb, in_=gmt_d[:, :])
            gamma_sb = persist.tile([P, 2], f32)
            nc.sync.dma_start(out=gamma_sb, in_=gamma_d[:, :])
            beta_sb = persist.tile([P, 2], f32)
            nc.sync.dma_start(out=beta_sb, in_=beta_d[:, :])
            bqk_sb = persist.tile([P, 4], f32)
            nc.sync.dma_start(out=bqk_sb, in_=bqk_d[:, :])
            bv_sb = persist.tile([P, 2], f32)
            nc.sync.dma_start(out=bv_sb, in_=bv_d[:, :])
            bp_sb = persist.tile([P, 2], f32)
            nc.sync.dma_start(out=bp_sb, in_=bp_d[:, :])
            b1_sb = persist.tile([64, 1], f32)
            nc.sync.dma_start(out=b1_sb, in_=b1_d[:, :])
            b2_sb = persist.tile([P, 2], f32)
            nc.sync.dma_start(out=b2_sb, in_=b2_d[:, :])

            x_sb = persist.tile([P, CT, BL, N], f32)

            def load_x(b):
                for ct in range(CT):
                    nc.sync.dma_start(out=x_sb[:, ct, b],
                                      in_=x_d[b, ct * P:(ct + 1) * P, :])

            load_x(0)
            load_x(1)
            wqk_sb = persist.tile([P, 2, 512], bf16)
            nc.sync.dma_start(out=wqk_sb, in_=wqk_d[:, :, :])
            wv_sb = persist.tile([P, 2, C], bf16)
            nc.sync.dma_start(out=wv_sb, in_=wv_d[:, :, :])
            load_x(2)
            load_x(3)
            wp_sb = persist.tile([P, 2, C], bf16)
            nc.sync.dma_start(out=wp_sb, in_=wp_d[:, :, :])
            w1_sb = persist.tile([P, 2, 64], f32)
            nc.sync.dma_start(out=w1_sb, in_=w1_d[:, :, :])
            w2_sb = persist.tile([64, C], f32)
            nc.sync.dma_start(out=w2_sb, in_=w2_d[:, :])
            ones_sb = persist.tile([P, P], bf16)
            nc.sync.dma_start(out=ones_sb, in_=ones_d[:, :])

            eps_sb = persist.tile([16, 1], f32)
            nc.vector.memset(eps_sb, EPS)

            # ---- GroupNorm stats, pipelined per (sample, ctile) ----
            # scale/offset chains run vectorized over sample PAIRS so sample
            # 0's normalize only waits on samples 0-1, not all four.
            sums_c = persist.tile([P, CT, BL], f32)    # per-channel sums
            sumsq_c = persist.tile([P, CT, BL], f32)   # per-channel sum sq
            a_sb = persist.tile([P, CT, BL], f32)      # per-channel scale
            bb_sb = persist.tile([P, CT, BL], f32)     # per-channel offset
            xn_sb = persist.tile([P, CT, BL, N], bf16)
            for b in range(BL):
                for ct in range(CT):
                    nc.vector.reduce_sum(
                        out=sums_c[:, ct, b:b + 1], in_=x_sb[:, ct, b], axis=AX)
                    jt = junk_pool.tile([P, N], bf16, tag="junk")
                    nc.scalar.activation(
                        out=jt, in_=x_sb[:, ct, b], func=AF.Square,
                        accum_out=sumsq_c[:, ct, b:b + 1])
                if b % 2 == 1:
                    pr = slice(b - 1, b + 1)  # this sample pair
                    for ct in range(CT):
                        ps_g = psum_small.tile([16, 4], f32, tag="pss")
                        nc.tensor.matmul(ps_g[:, 0:2], gm_sb,
                                         sums_c[:, ct, pr],
                                         start=True, stop=True)
                        nc.tensor.matmul(ps_g[:, 2:4], gm_sb,
                                         sumsq_c[:, ct, pr],
                                         start=True, stop=True)
                        nmean = persist.tile([16, 2], f32)
                        nc.vector.tensor_scalar_mul(nmean, ps_g[:, 0:2],
                                                    -1.0 / (GSIZE * N))
                        var = persist.tile([16, 2], f32)
                        nc.vector.tensor_scalar_mul(var, ps_g[:, 2:4],
                                                    1.0 / (GSIZE * N))
                        msq = persist.tile([16, 2], f32)
                        nc.vector.tensor_mul(msq, nmean, nmean)
                        nc.vector.tensor_sub(var, var, msq)
                        sd = persist.tile([16, 2], f32)
                        nc.scalar.activation(out=sd, in_=var, func=AF.Sqrt,
                                             bias=eps_sb)
                        rsm = persist.tile([16, 4], f32)
                        nc.vector.reciprocal(rsm[:, 0:2], sd)
                        nc.vector.tensor_mul(rsm[:, 2:4], nmean, rsm[:, 0:2])
                        ps_bc = psum_small.tile([P, 4], f32, tag="pss")
                        nc.tensor.matmul(ps_bc, gmt_sb, rsm,
                                         start=True, stop=True)
                        nc.vector.tensor_scalar_mul(
                            a_sb[:, ct, pr], ps_bc[:, 0:2],
                            gamma_sb[:, ct:ct + 1])
                        nc.vector.tensor_scalar(
                            out=bb_sb[:, ct, pr], in0=ps_bc[:, 2:4],
                            scalar1=gamma_sb[:, ct:ct + 1],
                            scalar2=beta_sb[:, ct:ct + 1],
                            op0=ALU.mult, op1=ALU.add)
                    for bb in (b - 1, b):
                        for ct in range(CT):
                            nc.scalar.activation(
                                out=xn_sb[:, ct, bb], in_=x_sb[:, ct, bb],
                                func=AF.Identity,
                                bias=bb_sb[:, ct, bb:bb + 1],
                                scale=a_sb[:, ct, bb:bb + 1])

            # ---- SE gate (per core, channel means already in sums_c) ----
            ps_h1 = psum_small.tile([64, BL], f32, tag="pss")
            for ct in range(CT):
                nc.tensor.matmul(ps_h1, w1_sb[:, ct], sums_c[:, ct],
                                 start=(ct == 0), stop=(ct == 1))
            h1_sb = persist.tile([64, BL], f32)
            nc.scalar.activation(out=h1_sb, in_=ps_h1, func=AF.Relu,
                                 bias=b1_sb[:, 0:1], scale=1.0 / N)
            gate_sb = persist.tile([P, CT, BL], f32)
            for ot in range(CT):
                ps_gate = psum_small.tile([P, BL], f32, tag="pss")
                nc.tensor.matmul(ps_gate, w2_sb[:, ot * P:(ot + 1) * P], h1_sb,
                                 start=True, stop=True)
                nc.scalar.activation(out=gate_sb[:, ot], in_=ps_gate,
                                     func=AF.Sigmoid, bias=b2_sb[:, ot:ot + 1])

            # ---- per-sample attention ----
            for b in range(BL):
                # q, k : [c, n] layout. m-tile 0,1 = q rows; 2,3 = k rows
                qk_sb = qk_pool.tile([P, 4, N], bf16, tag="qk")
                for m in range(4):
                    ps_qk = psum_big.tile([P, N], f32, tag="psb")
                    for ns in range(2):
                        for kt in range(CT):
                            nc.tensor.matmul(
                                ps_qk[:, ns * 512:(ns + 1) * 512],
                                wqk_sb[:, kt, m * P:(m + 1) * P],
                                xn_sb[:, kt, b, ns * 512:(ns + 1) * 512],
                                start=(kt == 0), stop=(kt == 1))
                    nc.vector.tensor_scalar_add(qk_sb[:, m], ps_qk,
                                                bqk_sb[:, m:m + 1])

                # vT : [n, c] layout (j on partitions)
                vt_sb = vt_pool.tile([P, 8, C], bf16, tag="vt")
                for jt in range(8):
                    ps_vt = psum_small.tile([P, C], f32, tag="pss")
                    for kt in range(CT):
                        nc.tensor.matmul(
                            ps_vt,
                            xn_sb[:, kt, b, jt * P:(jt + 1) * P],
                            wv_sb[:, kt],
                            start=(kt == 0), stop=(kt == 1))
                    nc.scalar.activation(out=vt_sb[:, jt], in_=ps_vt,
                                         func=AF.Copy)

                # esT = exp(S^T / 16) : [j, i] layout
                es_sb = es_pool.tile([P, 8, N], bf16, tag="es")
                for mt in range(8):
                    ps_s = psum_big.tile([P, N], f32, tag="psb")
                    for ns in range(2):
                        for kt in range(CT):
                            nc.tensor.matmul(
                                ps_s[:, ns * 512:(ns + 1) * 512],
                                qk_sb[:, 2 + kt, mt * P:(mt + 1) * P],
                                qk_sb[:, kt, ns * 512:(ns + 1) * 512],
                                start=(kt == 0), stop=(kt == 1))
                    nc.scalar.activation(out=es_sb[:, mt], in_=ps_s,
                                         func=AF.Exp, scale=0.0625)

                # softmax denominators: pairwise tile tree on GpSimd
                # (frees PE: 2 ones-matmuls instead of 16), broadcast to
                # 128 partitions via the ones-matmul contraction.
                tr_sb = tree_pool.tile([P, 7, N], bf16, tag="tree")
                for t in range(4):
                    nc.gpsimd.tensor_add(tr_sb[:, t], es_sb[:, 2 * t],
                                         es_sb[:, 2 * t + 1])
                nc.gpsimd.tensor_add(tr_sb[:, 4], tr_sb[:, 0], tr_sb[:, 1])
                nc.gpsimd.tensor_add(tr_sb[:, 5], tr_sb[:, 2], tr_sb[:, 3])
                nc.gpsimd.tensor_add(tr_sb[:, 6], tr_sb[:, 4], tr_sb[:, 5])
                ps_sum = psum_big.tile([P, N], f32, tag="psb")
                for ns in range(2):
                    nc.tensor.matmul(
                        ps_sum[:, ns * 512:(ns + 1) * 512],
                        ones_sb,
                        tr_sb[:, 6, ns * 512:(ns + 1) * 512],
                        start=True, stop=True)
                r_sb = r_pool.tile([P, N], f32, tag="rr")
                nc.vector.reciprocal_approx_fast(out=r_sb, in_=ps_sum)

                # AV (unnormalized) then column-scale by r
                xat_sb = xat_pool.tile([P, CT, N], bf16, tag="xat")
                for ct2 in range(CT):
                    ps_av = psum_big.tile([P, N], f32, tag="psb")
                    for ns in range(2):
                        for jt in range(8):
                            nc.tensor.matmul(
                                ps_av[:, ns * 512:(ns + 1) * 512],
                                vt_sb[:, jt, ct2 * P:(ct2 + 1) * P],
                                es_sb[:, jt, ns * 512:(ns + 1) * 512],
                                start=(jt == 0), stop=(jt == 7))
                    if want_bias_v:
                        tmp = r_pool.tile([P, N], f32, tag="avtmp")
                        nc.vector.tensor_mul(tmp, ps_av, r_sb)
                        nc.vector.tensor_scalar_add(xat_sb[:, ct2], tmp,
                                                    bv_sb[:, ct2:ct2 + 1])
                    else:
                        nc.vector.tensor_mul(xat_sb[:, ct2], ps_av, r_sb)

                # proj + SE gate + residual
                for ot in range(CT):
                    ps_y = psum_big.tile([P, N], f32, tag="psb")
                    for ns in range(2):
                        for kt2 in range(CT):
                            nc.tensor.matmul(
                                ps_y[:, ns * 512:(ns + 1) * 512],
                                wp_sb[:, kt2, ot * P:(ot + 1) * P],
                                xat_sb[:, kt2, ns * 512:(ns + 1) * 512],
                                start=(kt2 == 0), stop=(kt2 == 1))
                    out_t = out_pool.tile([P, N], f32, tag="outp")
                    for h in range(2):
                        hs = slice(h * 512, (h + 1) * 512)
                        nc.vector.tensor_scalar(
                            out=out_t[:, hs], in0=ps_y[:, hs],
                            scalar1=bp_sb[:, ot:ot + 1],
                            scalar2=gate_sb[:, ot, b:b + 1],
                            op0=ALU.add, op1=ALU.mult)
                        nc.vector.tensor_add(out_t[:, hs], out_t[:, hs],
                                             x_sb[:, ot, b, hs])
                        nc.sync.dma_start(
                            out=out_d[b, ot * P:(ot + 1) * P, hs],
                            in_=out_t[:, hs])

    nc.compile()
    return nc


def _prep_inputs(x, gn_gamma, gn_beta, w_qkv, b_qkv, w_proj, b_proj,
                 w_se1, b_se1, w_se2, b_se2):
    bf = ml_dtypes.bfloat16
    f32 = np.float32

    def pt(w):  # [K, M] -> [128, K//128, M] partition-tiled
        K, M = w.shape
        return np.ascontiguousarray(w.reshape(K // P, P, M).transpose(1, 0, 2))

    wqk = pt(np.ascontiguousarray(w_qkv[:512].T)).astype(bf)       # [128,2,512]
    wv = pt(np.ascontiguousarray(w_qkv[512:].T)).astype(bf)        # [128,2,256]
    wp = pt(np.ascontiguousarray(w_proj.T)).astype(bf)             # [128,2,256]
    w1 = pt(np.ascontiguousarray(w_se1.T)).astype(f32)             # [128,2,64]
    w2 = np.ascontiguousarray(w_se2.T).astype(f32)                 # [64,256]

    def pcol(v):  # [256] -> [128, 2]
        return np.ascontiguousarray(v.reshape(2, P).T).astype(f32)

    gm = np.zeros((P, 16), f32)
    gm[np.arange(P), np.arange(P) // GSIZE] = 1.0
    shared = {
        "wqk": wqk, "wv": wv, "wp": wp, "w1": w1, "w2": w2,
        "gamma": pcol(gn_gamma), "beta": pcol(gn_beta),
        "bqk": np.ascontiguousarray(b_qkv[:512].reshape(4, P).T).astype(f32),
        "bv": pcol(b_qkv[512:]), "bp": pcol(b_proj),
        "b1": np.asarray(b_se1, f32).reshape(64, 1),
        "b2": pcol(b_se2),
        "gm": gm, "gmt": np.ascontiguousarray(gm.T),
        "ones": np.ones((P, P), bf),
    }
    xr = np.asarray(x, f32).reshape(B, C, N)
    in_maps = []
    for i in range(NCORES):
        m = dict(shared)
        m["x"] = np.ascontiguousarray(xr[i * BL:(i + 1) * BL])
        in_maps.append(m)
    want_bias_v = bool(np.any(np.asarray(b_qkv[512:]) != 0))
    return in_maps, want_bias_v


def _get_program(want_bias_v):
    key = ("prog", want_bias_v)
    if key not in _CACHE:
        _CACHE[key] = _build_program(want_bias_v)
    return _CACHE[key]


def run(inputs, trace=False, trace_kwargs=None):
    """Build + run on all 8 cores. Returns (full_out, BassKernelResults)."""
    from concourse.bass_utils import run_bass_kernel_spmd

    in_maps, want_bias_v = _prep_inputs(**inputs)
    nc = _get_program(want_bias_v)
    kw = {}
    if trace:
        kw["trace"] = True
        if trace_kwargs:
            kw["trace_kwargs"] = trace_kwargs
    res = run_bass_kernel_spmd(nc, in_maps, list(range(NCORES)), **kw)
    out = np.concatenate([res.results[i]["out"] for i in range(NCORES)], axis=0)
    return out.reshape(B, C, HW, HW).astype(np.float32), res


def kernel(**inputs):
    out, _ = run(inputs, trace=False)
    return out


# revision 12
# speedup vs baseline: 1.2419x; 1.2419x over previous
"""AttentionBlock (GroupNorm + single-head spatial attention + SE gate + residual)
Trainium2 Bass/Tile kernel, data-parallel over batch across 8 NeuronCores.

Full shapes: x [32, 256, 32, 32] f32 -> out [32, 256, 32, 32] f32.
Per core: 4 samples. Per sample (C=256, N=1024):
  xn = GroupNorm(x) (32 groups)            [C, N]  (bf16)
  q, k = Wqk @ xn                          [2C, N] (bf16, [c,n] layout)
  vT = xn^T @ WvT                          [N, C]  (bf16, [n,c] layout - direct!)
  esT = exp((k^T q) / 16)                  [N, N]  ([j, i] layout, j = softmax axis)
  sums_bc = ones128 @ esT  (accum over j)  [128, N] (each row = sum_j exp)
  r = 1/sums (reciprocal_approx_fast)      [128, N]
  xat = (vT^T @ esT) * r                   [C, N]  (unnormalized AV, scaled after)
  y = Wp @ xat                             [C, N]
  out = x + (y + bp) * gate[c]             (gate = SE sigmoid path from channel means)

No transposes anywhere: softmax reductions over j land on the PE contraction
axis (ones-matmul), the normalization is a rank-1 column scale folded in after
the AV matmul.
"""

import numpy as np
import ml_dtypes

B, C, HW, N = 32, 256, 32, 1024
NCORES = 8
BL = B // NCORES          # samples per core
GROUPS = 32
GSIZE = C // GROUPS       # 8 channels per group
EPS = 1e-5
CT = 2                    # channel partition tiles (256 = 2*128)
P = 128

_CACHE = {}


def _build_program(want_bias_v):
    import concourse.bacc as bacc
    import concourse.mybir as mybir
    import concourse.tile as tile

    f32 = mybir.dt.float32
    bf16 = mybir.dt.bfloat16
    AX = mybir.AxisListType.X
    AF = mybir.ActivationFunctionType
    ALU = mybir.AluOpType

    nc = bacc.Bacc()

    # ---- DRAM I/O ----
    x_d = nc.dram_tensor("x", [BL, C, N], f32, kind="ExternalInput")
    out_d = nc.dram_tensor("out", [BL, C, N], f32, kind="ExternalOutput")
    wqk_d = nc.dram_tensor("wqk", [P, 2, 512], bf16, kind="ExternalInput")
    wv_d = nc.dram_tensor("wv", [P, 2, C], bf16, kind="ExternalInput")
    wp_d = nc.dram_tensor("wp", [P, 2, C], bf16, kind="ExternalInput")
    w1_d = nc.dram_tensor("w1", [P, 2, 64], f32, kind="ExternalInput")
    w2_d = nc.dram_tensor("w2", [64, C], f32, kind="ExternalInput")
    gamma_d = nc.dram_tensor("gamma", [P, 2], f32, kind="ExternalInput")
    beta_d = nc.dram_tensor("beta", [P, 2], f32, kind="ExternalInput")
    bqk_d = nc.dram_tensor("bqk", [P, 4], f32, kind="ExternalInput")
    bv_d = nc.dram_tensor("bv", [P, 2], f32, kind="ExternalInput")
    bp_d = nc.dram_tensor("bp", [P, 2], f32, kind="ExternalInput")
    b1_d = nc.dram_tensor("b1", [64, 1], f32, kind="ExternalInput")
    b2_d = nc.dram_tensor("b2", [P, 2], f32, kind="ExternalInput")
    gm_d = nc.dram_tensor("gm", [P, 16], f32, kind="ExternalInput")
    gmt_d = nc.dram_tensor("gmt", [16, P], f32, kind="ExternalInput")
    ones_d = nc.dram_tensor("ones", [P, P], bf16, kind="ExternalInput")

    with tile.TileContext(nc) as tc:
        with (
            tc.tile_pool(name="persist", bufs=1) as persist,
            tc.tile_pool(name="qk", bufs=2) as qk_pool,
            tc.tile_pool(name="vt", bufs=2) as vt_pool,
            tc.tile_pool(name="es", bufs=2) as es_pool,
            tc.tile_pool(name="xat", bufs=2) as xat_pool,
            tc.tile_pool(name="rr", bufs=2) as r_pool,
            tc.tile_pool(name="junk", bufs=2) as junk_pool,
            tc.tile_pool(name="outp", bufs=3) as out_pool,
            tc.tile_pool(name="psb", bufs=3, space="PSUM") as psum_big,
            tc.tile_pool(name="pss", bufs=2, space="PSUM") as psum_small,
        ):
            # ---- DMA prologue: x slices first (sample 0's stats are the
            # critical path), then consts/weights in first-use order (the
            # HWDGE queue is FIFO in emission order).
            x_sb = persist.tile([P, CT, BL, N], f32)

            def load_x(b):
                for ct in range(CT):
                    nc.sync.dma_start(out=x_sb[:, ct, b],
                                      in_=x_d[b, ct * P:(ct + 1) * P, :])

            load_x(0)
            load_x(1)
            gm_sb = persist.tile([P, 16], f32)
            nc.sync.dma_start(out=gm_sb, in_=gm_d[:, :])
            gmt_sb = persist.tile([16, P], f32)
            nc.sync.dma_start(out=gmt_sb, in_=gmt_d[:, :])
            gamma_sb = persist.tile([P, 2], f32)
            nc.sync.dma_start(out=gamma_sb, in_=gamma_d[:, :])
            beta_sb = persist.tile([P, 2], f32)
            nc.sync.dma_start(out=beta_sb, in_=beta_d[:, :])
            bqk_sb = persist.tile([P, 4], f32)
            nc.sync.dma_start(out=bqk_sb, in_=bqk_d[:, :])
            bv_sb = persist.tile([P, 2], f32)
            nc.sync.dma_start(out=bv_sb, in_=bv_d[:, :])
            bp_sb = persist.tile([P, 2], f32)
            nc.sync.dma_start(out=bp_sb, in_=bp_d[:, :])
            b1_sb = persist.tile([64, 1], f32)
            nc.sync.dma_start(out=b1_sb, in_=b1_d[:, :])
            b2_sb = persist.tile([P, 2], f32)
            nc.sync.dma_start(out=b2_sb, in_=b2_d[:, :])
            wqk_sb = persist.tile([P, 2, 512], bf16)
            nc.sync.dma_start(out=wqk_sb, in_=wqk_d[:, :, :])
            wv_sb = persist.tile([P, 2, C], bf16)
            nc.sync.dma_start(out=wv_sb, in_=wv_d[:, :, :])
            load_x(2)
            load_x(3)
            wp_sb = persist.tile([P, 2, C], bf16)
            nc.sync.dma_start(out=wp_sb, in_=wp_d[:, :, :])
            w1_sb = persist.tile([P, 2, 64], f32)
            nc.sync.dma_start(out=w1_sb, in_=w1_d[:, :, :])
            w2_sb = persist.tile([64, C], f32)
            nc.sync.dma_start(out=w2_sb, in_=w2_d[:, :])
            ones_sb = persist.tile([P, P], bf16)
            nc.sync.dma_start(out=ones_sb, in_=ones_d[:, :])

            eps_sb = persist.tile([16, 1], f32)
            nc.vector.memset(eps_sb, EPS)

            # ---- GroupNorm stats, pipelined per (sample, ctile) ----
            # scale/offset chains run vectorized over sample PAIRS so sample
            # 0's normalize only waits on samples 0-1, not all four.
            sums_c = persist.tile([P, CT, BL], f32)    # per-channel sums
            sumsq_c = persist.tile([P, CT, BL], f32)   # per-channel sum sq
            a_sb = persist.tile([P, CT, BL], f32)      # per-channel scale
            bb_sb = persist.tile([P, CT, BL], f32)     # per-channel offset
            xn_sb = persist.tile([P, CT, BL, N], bf16)
            for b in range(BL):
                for ct in range(CT):
                    nc.vector.reduce_sum(
                        out=sums_c[:, ct, b:b + 1], in_=x_sb[:, ct, b], axis=AX)
                    jt = junk_pool.tile([P, N], bf16, tag="junk")
                    nc.scalar.activation(
                        out=jt, in_=x_sb[:, ct, b], func=AF.Square,
                        accum_out=sumsq_c[:, ct, b:b + 1])
                if b % 2 == 1:
                    pr = slice(b - 1, b + 1)  # this sample pair
                    for ct in range(CT):
                        ps_g = psum_small.tile([16, 4], f32, tag="pss")
                        nc.tensor.matmul(ps_g[:, 0:2], gm_sb,
                                         sums_c[:, ct, pr],
                                         start=True, stop=True)
                        nc.tensor.matmul(ps_g[:, 2:4], gm_sb,
                                         sumsq_c[:, ct, pr],
                                         start=True, stop=True)
                        nmean = persist.tile([16, 2], f32)
                        nc.vector.tensor_scalar_mul(nmean, ps_g[:, 0:2],
                                                    -1.0 / (GSIZE * N))
                        var = persist.tile([16, 2], f32)
                        nc.vector.tensor_scalar_mul(var, ps_g[:, 2:4],
                                                    1.0 / (GSIZE * N))
                        msq = persist.tile([16, 2], f32)
                        nc.vector.tensor_mul(msq, nmean, nmean)
                        nc.vector.tensor_sub(var, var, msq)
                        sd = persist.tile([16, 2], f32)
                        nc.scalar.activation(out=sd, in_=var, func=AF.Sqrt,
                                             bias=eps_sb)
                        rsm = persist.tile([16, 4], f32)
                        nc.vector.reciprocal(rsm[:, 0:2], sd)
                        nc.vector.tensor_mul(rsm[:, 2:4], nmean, rsm[:, 0:2])
                        ps_bc = psum_small.tile([P, 4], f32, tag="pss")
                        nc.tensor.matmul(ps_bc, gmt_sb, rsm,
                                         start=True, stop=True)
                        nc.vector.tensor_scalar_mul(
                            a_sb[:, ct, pr], ps_bc[:, 0:2],
                            gamma_sb[:, ct:ct + 1])
                        nc.vector.tensor_scalar(
                            out=bb_sb[:, ct, pr], in0=ps_bc[:, 2:4],
                            scalar1=gamma_sb[:, ct:ct + 1],
                            scalar2=beta_sb[:, ct:ct + 1],
                            op0=ALU.mult, op1=ALU.add)
                    for bb in (b - 1, b):
                        for ct in range(CT):
                            nc.scalar.activation(
                                out=xn_sb[:, ct, bb], in_=x_sb[:, ct, bb],
                                func=AF.Identity,
                                bias=bb_sb[:, ct, bb:bb + 1],
                                scale=a_sb[:, ct, bb:bb + 1])

            # ---- SE gate (per core, channel means already in sums_c) ----
            ps_h1 = psum_small.tile([64, BL], f32, tag="pss")
            for ct in range(CT):
                nc.tensor.matmul(ps_h1, w1_sb[:, ct], sums_c[:, ct],
                                 start=(ct == 0), stop=(ct == 1))
            h1_sb = persist.tile([64, BL], f32)
            nc.scalar.activation(out=h1_sb, in_=ps_h1, func=AF.Relu,
                                 bias=b1_sb[:, 0:1], scale=1.0 / N)
            gate_sb = persist.tile([P, CT, BL], f32)
            for ot in range(CT):
                ps_gate = psum_small.tile([P, BL], f32, tag="pss")
                nc.tensor.matmul(ps_gate, w2_sb[:, ot * P:(ot + 1) * P], h1_sb,
                                 start=True, stop=True)
                nc.scalar.activation(out=gate_sb[:, ot], in_=ps_gate,
                                     func=AF.Sigmoid, bias=b2_sb[:, ot:ot + 1])

            # ---- per-sample attention ----
            for b in range(BL):
                # q, k : [c, n] layout. m-tile 0,1 = q rows; 2,3 = k rows
                qk_sb = qk_pool.tile([P, 4, N], bf16, tag="qk")
                for m in range(4):
                    ps_qk = psum_big.tile([P, N], f32, tag="psb")
                    for ns in range(2):
                        for kt in range(CT):
                            nc.tensor.matmul(
                                ps_qk[:, ns * 512:(ns + 1) * 512],
                                wqk_sb[:, kt, m * P:(m + 1) * P],
                                xn_sb[:, kt, b, ns * 512:(ns + 1) * 512],
                                start=(kt == 0), stop=(kt == 1))
                    nc.vector.tensor_scalar_add(qk_sb[:, m], ps_qk,
                                                bqk_sb[:, m:m + 1])

                # vT : [n, c] layout (j on partitions)
                vt_sb = vt_pool.tile([P, 8, C], bf16, tag="vt")
                for jt in range(8):
                    ps_vt = psum_small.tile([P, C], f32, tag="pss")
                    for kt in range(CT):
                        nc.tensor.matmul(
                            ps_vt,
                            xn_sb[:, kt, b, jt * P:(jt + 1) * P],
                            wv_sb[:, kt],
                            start=(kt == 0), stop=(kt == 1))
                    nc.scalar.activation(out=vt_sb[:, jt], in_=ps_vt,
                                         func=AF.Copy)

                # esT = exp(S^T / 16) : [j, i] layout
                es_sb = es_pool.tile([P, 8, N], bf16, tag="es")
                for mt in range(8):
                    ps_s = psum_big.tile([P, N], f32, tag="psb")
                    for ns in range(2):
                        for kt in range(CT):
                            nc.tensor.matmul(
                                ps_s[:, ns * 512:(ns + 1) * 512],
                                qk_sb[:, 2 + kt, mt * P:(mt + 1) * P],
                                qk_sb[:, kt, ns * 512:(ns + 1) * 512],
                                start=(kt == 0), stop=(kt == 1))
                    nc.scalar.activation(out=es_sb[:, mt], in_=ps_s,
                                         func=AF.Exp, scale=0.0625)

                # softmax denominators, broadcast to 128 partitions
                ps_sum = psum_big.tile([P, N], f32, tag="psb")
                for ns in range(2):
                    for jt in range(8):
                        nc.tensor.matmul(
                            ps_sum[:, ns * 512:(ns + 1) * 512],
                            ones_sb,
                            es_sb[:, jt, ns * 512:(ns + 1) * 512],
                            start=(jt == 0), stop=(jt == 7))
                r_sb = r_pool.tile([P, N], f32, tag="rr")
                nc.vector.reciprocal_approx_fast(out=r_sb, in_=ps_sum)

                # AV (unnormalized) then column-scale by r
                xat_sb = xat_pool.tile([P, CT, N], bf16, tag="xat")
                for ct2 in range(CT):
                    ps_av = psum_big.tile([P, N], f32, tag="psb")
                    for ns in range(2):
                        for jt in range(8):
                            nc.tensor.matmul(
                                ps_av[:, ns * 512:(ns + 1) * 512],
                                vt_sb[:, jt, ct2 * P:(ct2 + 1) * P],
                                es_sb[:, jt, ns * 512:(ns + 1) * 512],
                                start=(jt == 0), stop=(jt == 7))
                    if want_bias_v:
                        tmp = r_pool.tile([P, N], f32, tag="avtmp")
                        nc.vector.tensor_mul(tmp, ps_av, r_sb)
                        nc.vector.tensor_scalar_add(xat_sb[:, ct2], tmp,
                                                    bv_sb[:, ct2:ct2 + 1])
                    else:
                        nc.vector.tensor_mul(xat_sb[:, ct2], ps_av, r_sb)

                # proj + SE gate + residual
                for ot in range(CT):
                    ps_y = psum_big.tile([P, N], f32, tag="psb")
                    for ns in range(2):
                        for kt2 in range(CT):
                            nc.tensor.matmul(
                                ps_y[:, ns * 512:(ns + 1) * 512],
                                wp_sb[:, kt2, ot * P:(ot + 1) * P],
                                xat_sb[:, kt2, ns * 512:(ns + 1) * 512],
                                start=(kt2 == 0), stop=(kt2 == 1))
                    out_t = out_pool.tile([P, N], f32, tag="outp")
                    for h in range(2):
                        hs = slice(h * 512, (h + 1) * 512)
                        nc.vector.tensor_scalar(
                            out=out_t[:, hs], in0=ps_y[:, hs],
                            scalar1=bp_sb[:, ot:ot + 1],
                            scalar2=gate_sb[:, ot, b:b + 1],
                            op0=ALU.add, op1=ALU.mult)
                        nc.vector.tensor_add(out_t[:, hs], out_t[:, hs],
                                             x_sb[:, ot, b, hs])
                        nc.sync.dma_start(
                            out=out_d[b, ot * P:(ot + 1) * P, hs],
                            in_=out_t[:, hs])

    nc.compile()
    return nc


def _prep_inputs(x, gn_gamma, gn_beta, w_qkv, b_qkv, w_proj, b_proj,
                 w_se1, b_se1, w_se2, b_se2):
    bf = ml_dtypes.bfloat16
    f32 = np.float32

    def pt(w):  # [K, M] -> [128, K//128, M] partition-tiled
        K, M = w.shape
        return np.ascontiguousarray(w.reshape(K // P, P, M).transpose(1, 0, 2))

    wqk = pt(np.ascontiguousarray(w_qkv[:512].T)).astype(bf)       # [128,2,512]
    wv = pt(np.ascontiguousarray(w_qkv[512:].T)).astype(bf)        # [128,2,256]
    wp = pt(np.ascontiguousarray(w_proj.T)).astype(bf)             # [128,2,256]
    w1 = pt(np.ascontiguousarray(w_se1.T)).astype(f32)             # [128,2,64]
    w2 = np.ascontiguousarray(w_se2.T).astype(f32)                 # [64,256]

    def pcol(v):  # [256] -> [128, 2]
        return np.ascontiguousarray(v.reshape(2, P).T).astype(f32)

    gm = np.zeros((P, 16), f32)
    gm[np.arange(P), np.arange(P) // GSIZE] = 1.0
    shared = {
        "wqk": wqk, "wv": wv, "wp": wp, "w1": w1, "w2": w2,
        "gamma": pcol(gn_gamma), "beta": pcol(gn_beta),
        "bqk": np.ascontiguousarray(b_qkv[:512].reshape(4, P).T).astype(f32),
        "bv": pcol(b_qkv[512:]), "bp": pcol(b_proj),
        "b1": np.asarray(b_se1, f32).reshape(64, 1),
        "b2": pcol(b_se2),
        "gm": gm, "gmt": np.ascontiguousarray(gm.T),
        "ones": np.ones((P, P), bf),
    }
    xr = np.asarray(x, f32).reshape(B, C, N)
    in_maps = []
    for i in range(NCORES):
        m = dict(shared)
        m["x"] = np.ascontiguousarray(xr[i * BL:(i + 1) * BL])
        in_maps.append(m)
    want_bias_v = bool(np.any(np.asarray(b_qkv[512:]) != 0))
    return in_maps, want_bias_v


def _get_program(want_bias_v):
    key = ("prog", want_bias_v)
    if key not in _CACHE:
        _CACHE[key] = _build_program(want_bias_v)
    return _CACHE[key]


def run(inputs, trace=False, trace_kwargs=None):
    """Build + run on all 8 cores. Returns (full_out, BassKernelResults)."""
    from concourse.bass_utils import run_bass_kernel_spmd

    in_maps, want_bias_v = _prep_inputs(**inputs)
    nc = _get_program(want_bias_v)
    kw = {}
    if trace:
        kw["trace"] = True
        if trace_kwargs:
            kw["trace_kwargs"] = trace_kwargs
    res = run_bass_kernel_spmd(nc, in_maps, list(range(NCORES)), **kw)
    out = np.concatenate([res.results[i]["out"] for i in range(NCORES)], axis=0)
    return out.reshape(B, C, HW, HW).astype(np.float32), res


def kernel(**inputs):
    out, _ = run(inputs, trace=False)
    return out


# revision 14
# speedup vs baseline: 1.4119x; 1.1369x over previous
"""AttentionBlock (GroupNorm + single-head spatial attention + SE gate + residual)
Trainium2 Bass/Tile kernel, data-parallel over batch across 8 NeuronCores.

Full shapes: x [32, 256, 32, 32] f32 -> out [32, 256, 32, 32] f32.
Per core: 4 samples. Per sample (C=256, N=1024):
  xn = GroupNorm(x) (32 groups)            [C, N]  (bf16)
  q, k = Wqk @ xn                          [2C, N] (bf16, [c,n] layout)
  vT = xn^T @ WvT                          [N, C]  (bf16, [n,c] layout - direct!)
  esT = exp((k^T q) / 16)                  [N, N]  ([j, i] layout, j = softmax axis)
  sums_bc = ones128 @ esT  (accum over j)  [128, N] (each row = sum_j exp)
  r = 1/sums (reciprocal_approx_fast)      [128, N]
  xat = (vT^T @ esT) * r                   [C, N]  (unnormalized AV, scaled after)
  y = Wp @ xat                             [C, N]
  out = x + (y + bp) * gate[c]             (gate = SE sigmoid path from channel means)

No transposes anywhere: softmax reductions over j land on the PE contraction
axis (ones-matmul), the normalization is a rank-1 column scale folded in after
the AV matmul.
"""

import numpy as np
import ml_dtypes

B, C, HW, N = 32, 256, 32, 1024
NCORES = 8
BL = B // NCORES          # samples per core
GROUPS = 32
GSIZE = C // GROUPS       # 8 channels per group
EPS = 1e-5
CT = 2                    # channel partition tiles (256 = 2*128)
P = 128

_CACHE = {}


def _build_program(want_bias_v):
    import concourse.bacc as bacc
    import concourse.mybir as mybir
    import concourse.tile as tile

    f32 = mybir.dt.float32
    bf16 = mybir.dt.bfloat16
    AX = mybir.AxisListType.X
    AF = mybir.ActivationFunctionType
    ALU = mybir.AluOpType

    nc = bacc.Bacc()

    # ---- DRAM I/O ----
    x_d = nc.dram_tensor("x", [BL, C, N], f32, kind="ExternalInput")
    out_d = nc.dram_tensor("out", [BL, C, N], f32, kind="ExternalOutput")
    wqk_d = nc.dram_tensor("wqk", [P, 2, 512], bf16, kind="ExternalInput")
    wv_d = nc.dram_tensor("wv", [P, 2, C], bf16, kind="ExternalInput")
    wp_d = nc.dram_tensor("wp", [P, 2, C], bf16, kind="ExternalInput")
    w1_d = nc.dram_tensor("w1", [P, 2, 64], f32, kind="ExternalInput")
    w2_d = nc.dram_tensor("w2", [64, C], f32, kind="ExternalInput")
    gamma_d = nc.dram_tensor("gamma", [P, 2], f32, kind="ExternalInput")
    beta_d = nc.dram_tensor("beta", [P, 2], f32, kind="ExternalInput")
    bqk_d = nc.dram_tensor("bqk", [P, 4], f32, kind="ExternalInput")
    bv_d = nc.dram_tensor("bv", [P, 2], f32, kind="ExternalInput")
    bp_d = nc.dram_tensor("bp", [P, 2], f32, kind="ExternalInput")
    b1_d = nc.dram_tensor("b1", [64, 1], f32, kind="ExternalInput")
    b2_d = nc.dram_tensor("b2", [P, 2], f32, kind="ExternalInput")
    gm_d = nc.dram_tensor("gm", [P, 16], f32, kind="ExternalInput")
    gmt_d = nc.dram_tensor("gmt", [16, P], f32, kind="ExternalInput")
    ones_d = nc.dram_tensor("ones", [P, P], bf16, kind="ExternalInput")

    with tile.TileContext(nc) as tc:
        with (
            tc.tile_pool(name="persist", bufs=1) as persist,
            tc.tile_pool(name="qk", bufs=2) as qk_pool,
            tc.tile_pool(name="vt", bufs=2) as vt_pool,
            tc.tile_pool(name="es", bufs=2) as es_pool,
            tc.tile_pool(name="xat", bufs=2) as xat_pool,
            tc.tile_pool(name="rr", bufs=2) as r_pool,
            tc.tile_pool(name="junk", bufs=2) as junk_pool,
            tc.tile_pool(name="outp", bufs=3) as out_pool,
            tc.tile_pool(name="psb", bufs=3, space="PSUM") as psum_big,
            tc.tile_pool(name="pss", bufs=2, space="PSUM") as psum_small,
        ):
            # ---- DMA prologue: x slices first (sample 0's stats are the
            # critical path), then consts/weights in first-use order (the
            # HWDGE queue is FIFO in emission order).
            x_sb = persist.tile([P, CT, BL, N], f32)

            def load_x(b):
                for ct in range(CT):
                    nc.sync.dma_start(out=x_sb[:, ct, b],
                                      in_=x_d[b, ct * P:(ct + 1) * P, :])

            load_x(0)
            load_x(1)
            gm_sb = persist.tile([P, 16], f32)
            nc.sync.dma_start(out=gm_sb, in_=gm_d[:, :])
            gmt_sb = persist.tile([16, P], f32)
            nc.sync.dma_start(out=gmt_sb, in_=gmt_d[:, :])
            gamma_sb = persist.tile([P, 2], f32)
            nc.sync.dma_start(out=gamma_sb, in_=gamma_d[:, :])
            beta_sb = persist.tile([P, 2], f32)
            nc.sync.dma_start(out=beta_sb, in_=beta_d[:, :])
            bqk_sb = persist.tile([P, 4], f32)
            nc.sync.dma_start(out=bqk_sb, in_=bqk_d[:, :])
            bv_sb = persist.tile([P, 2], f32)
            nc.sync.dma_start(out=bv_sb, in_=bv_d[:, :])
            bp_sb = persist.tile([P, 2], f32)
            nc.sync.dma_start(out=bp_sb, in_=bp_d[:, :])
            b1_sb = persist.tile([64, 1], f32)
            nc.sync.dma_start(out=b1_sb, in_=b1_d[:, :])
            b2_sb = persist.tile([P, 2], f32)
            nc.sync.dma_start(out=b2_sb, in_=b2_d[:, :])
            wqk_sb = persist.tile([P, 2, 512], bf16)
            nc.sync.dma_start(out=wqk_sb, in_=wqk_d[:, :, :])
            wv_sb = persist.tile([P, 2, C], bf16)
            nc.sync.dma_start(out=wv_sb, in_=wv_d[:, :, :])
            load_x(2)
            load_x(3)
            w1_sb = persist.tile([P, 2, 64], f32)
            nc.sync.dma_start(out=w1_sb, in_=w1_d[:, :, :])
            w2_sb = persist.tile([64, C], f32)
            nc.sync.dma_start(out=w2_sb, in_=w2_d[:, :])
            ones_sb = persist.tile([P, P], bf16)
            nc.sync.dma_start(out=ones_sb, in_=ones_d[:, :])
            wp_sb = persist.tile([P, 2, C], bf16)
            nc.sync.dma_start(out=wp_sb, in_=wp_d[:, :, :])

            eps_sb = persist.tile([16, 1], f32)
            nc.vector.memset(eps_sb, EPS)

            # ---- persistent intermediates ----
            sums_c = persist.tile([P, CT, BL], f32)    # per-channel sums
            sumsq_c = persist.tile([P, CT, BL], f32)   # per-channel sum sq
            a_sb = persist.tile([P, CT, BL], f32)      # per-channel scale
            bb_sb = persist.tile([P, CT, BL], f32)     # per-channel offset
            xn_sb = persist.tile([P, CT, BL, N], bf16)
            gate_sb = persist.tile([P, CT, BL], f32)
            bpg_sb = persist.tile([P, CT, BL], f32)    # bp * gate (ACT fuse bias)
            h1_sb = persist.tile([64, BL], f32)
            qk_tiles = [None] * BL

            def emit_stats(b):
                for ct in range(CT):
                    nc.vector.reduce_sum(
                        out=sums_c[:, ct, b:b + 1], in_=x_sb[:, ct, b], axis=AX)
                    jt = junk_pool.tile([P, N], bf16, tag="junk")
                    nc.scalar.activation(
                        out=jt, in_=x_sb[:, ct, b], func=AF.Square,
                        accum_out=sumsq_c[:, ct, b:b + 1])

            def emit_gn_pair(p):
                pr = slice(2 * p, 2 * p + 2)
                for ct in range(CT):
                    ps_g = psum_small.tile([16, 4], f32, tag="pss")
                    nc.tensor.matmul(ps_g[:, 0:2], gm_sb, sums_c[:, ct, pr],
                                     start=True, stop=True)
                    nc.tensor.matmul(ps_g[:, 2:4], gm_sb, sumsq_c[:, ct, pr],
                                     start=True, stop=True)
                    nmean = persist.tile([16, 2], f32)
                    nc.vector.tensor_scalar_mul(nmean, ps_g[:, 0:2],
                                                -1.0 / (GSIZE * N))
                    var = persist.tile([16, 2], f32)
                    nc.vector.tensor_scalar_mul(var, ps_g[:, 2:4],
                                                1.0 / (GSIZE * N))
                    msq = persist.tile([16, 2], f32)
                    nc.vector.tensor_mul(msq, nmean, nmean)
                    nc.vector.tensor_sub(var, var, msq)
                    sd = persist.tile([16, 2], f32)
                    nc.scalar.activation(out=sd, in_=var, func=AF.Sqrt,
                                         bias=eps_sb)
                    rsm = persist.tile([16, 4], f32)
                    nc.vector.reciprocal(rsm[:, 0:2], sd)
                    nc.vector.tensor_mul(rsm[:, 2:4], nmean, rsm[:, 0:2])
                    ps_bc = psum_small.tile([P, 4], f32, tag="pss")
                    nc.tensor.matmul(ps_bc, gmt_sb, rsm, start=True, stop=True)
                    nc.vector.tensor_scalar_mul(a_sb[:, ct, pr], ps_bc[:, 0:2],
                                                gamma_sb[:, ct:ct + 1])
                    nc.vector.tensor_scalar(
                        out=bb_sb[:, ct, pr], in0=ps_bc[:, 2:4],
                        scalar1=gamma_sb[:, ct:ct + 1],
                        scalar2=beta_sb[:, ct:ct + 1],
                        op0=ALU.mult, op1=ALU.add)
                for bb in (2 * p, 2 * p + 1):
                    for ct in range(CT):
                        nc.scalar.activation(
                            out=xn_sb[:, ct, bb], in_=x_sb[:, ct, bb],
                            func=AF.Identity,
                            bias=bb_sb[:, ct, bb:bb + 1],
                            scale=a_sb[:, ct, bb:bb + 1])

            def emit_se_pair(p):
                pr = slice(2 * p, 2 * p + 2)
                ps_h1 = psum_small.tile([64, 2], f32, tag="pss")
                for ct in range(CT):
                    nc.tensor.matmul(ps_h1, w1_sb[:, ct], sums_c[:, ct, pr],
                                     start=(ct == 0), stop=(ct == 1))
                nc.scalar.activation(out=h1_sb[:, pr], in_=ps_h1, func=AF.Relu,
                                     bias=b1_sb[:, 0:1], scale=1.0 / N)
                for ot in range(CT):
                    ps_gate = psum_small.tile([P, 2], f32, tag="pss")
                    nc.tensor.matmul(ps_gate, w2_sb[:, ot * P:(ot + 1) * P],
                                     h1_sb[:, pr], start=True, stop=True)
                    nc.scalar.activation(out=gate_sb[:, ot, pr], in_=ps_gate,
                                         func=AF.Sigmoid,
                                         bias=b2_sb[:, ot:ot + 1])
                    nc.vector.tensor_scalar_mul(bpg_sb[:, ot, pr],
                                                gate_sb[:, ot, pr],
                                                bp_sb[:, ot:ot + 1])

            def emit_qk(b):
                # q, k : [c, n] layout. m-tile 0,1 = q rows; 2,3 = k rows.
                # Evac order q0,k0,q1,k1 so S's first K-step unblocks early.
                qk_sb = qk_pool.tile([P, 4, N], bf16, tag="qk")
                qk_tiles[b] = qk_sb
                for m in (0, 2, 1, 3):
                    ps_qk = psum_big.tile([P, N], f32, tag="psb")
                    for ns in range(2):
                        for kt in range(CT):
                            nc.tensor.matmul(
                                ps_qk[:, ns * 512:(ns + 1) * 512],
                                wqk_sb[:, kt, m * P:(m + 1) * P],
                                xn_sb[:, kt, b, ns * 512:(ns + 1) * 512],
                                start=(kt == 0), stop=(kt == 1))
                    nc.vector.tensor_scalar_add(qk_sb[:, m], ps_qk,
                                                bqk_sb[:, m:m + 1])

            def emit_vt(b):
                vt_sb = vt_pool.tile([P, 8, C], bf16, tag="vt")
                for jt in range(8):
                    ps_vt = psum_small.tile([P, C], f32, tag="pss")
                    for kt in range(CT):
                        nc.tensor.matmul(
                            ps_vt,
                            xn_sb[:, kt, b, jt * P:(jt + 1) * P],
                            wv_sb[:, kt],
                            start=(kt == 0), stop=(kt == 1))
                    nc.scalar.activation(out=vt_sb[:, jt], in_=ps_vt,
                                         func=AF.Copy)
                return vt_sb

            def emit_s(b):
                qk_sb = qk_tiles[b]
                es_sb = es_pool.tile([P, 8, N], bf16, tag="es")
                for mt in range(8):
                    ps_s = psum_big.tile([P, N], f32, tag="psb")
                    for ns in range(2):
                        for kt in range(CT):
                            nc.tensor.matmul(
                                ps_s[:, ns * 512:(ns + 1) * 512],
                                qk_sb[:, 2 + kt, mt * P:(mt + 1) * P],
                                qk_sb[:, kt, ns * 512:(ns + 1) * 512],
                                start=(kt == 0), stop=(kt == 1))
                    nc.scalar.activation(out=es_sb[:, mt], in_=ps_s,
                                         func=AF.Exp, scale=0.0625)
                return es_sb

            def emit_sums(es_sb):
                ps_sum = psum_big.tile([P, N], f32, tag="psb")
                for ns in range(2):
                    for jt in range(8):
                        nc.tensor.matmul(
                            ps_sum[:, ns * 512:(ns + 1) * 512],
                            ones_sb,
                            es_sb[:, jt, ns * 512:(ns + 1) * 512],
                            start=(jt == 0), stop=(jt == 7))
                r_sb = r_pool.tile([P, N], f32, tag="rr")
                nc.vector.reciprocal_approx_fast(out=r_sb, in_=ps_sum)
                return r_sb

            def emit_av(b, vt_sb, es_sb, r_sb):
                xat_sb = xat_pool.tile([P, CT, N], bf16, tag="xat")
                for ct2 in range(CT):
                    ps_av = psum_big.tile([P, N], f32, tag="psb")
                    for ns in range(2):
                        for jt in range(8):
                            nc.tensor.matmul(
                                ps_av[:, ns * 512:(ns + 1) * 512],
                                vt_sb[:, jt, ct2 * P:(ct2 + 1) * P],
                                es_sb[:, jt, ns * 512:(ns + 1) * 512],
                                start=(jt == 0), stop=(jt == 7))
                    if want_bias_v:
                        tmp = r_pool.tile([P, N], f32, tag="avtmp")
                        nc.vector.tensor_mul(tmp, ps_av, r_sb)
                        nc.vector.tensor_scalar_add(xat_sb[:, ct2], tmp,
                                                    bv_sb[:, ct2:ct2 + 1])
                    else:
                        nc.vector.tensor_mul(xat_sb[:, ct2], ps_av, r_sb)
                return xat_sb

            def emit_proj_fuse(b, xat_sb, act_scale):
                for ot in range(CT):
                    ps_y = psum_big.tile([P, N], f32, tag="psb")
                    for ns in range(2):
                        for kt2 in range(CT):
                            nc.tensor.matmul(
                                ps_y[:, ns * 512:(ns + 1) * 512],
                                wp_sb[:, kt2, ot * P:(ot + 1) * P],
                                xat_sb[:, kt2, ns * 512:(ns + 1) * 512],
                                start=(kt2 == 0), stop=(kt2 == 1))
                    out_t = out_pool.tile([P, N], f32, tag="outp")
                    for h in range(2):
                        hs = slice(h * 512, (h + 1) * 512)
                        if act_scale:
                            # (y + bp)*gate = y*gate + bp*gate on ScalarE,
                            # freeing VectorE for the residual adds at the tail
                            nc.scalar.activation(
                                out=out_t[:, hs], in_=ps_y[:, hs],
                                func=AF.Identity,
                                bias=bpg_sb[:, ot, b:b + 1],
                                scale=gate_sb[:, ot, b:b + 1])
                        else:
                            nc.vector.tensor_scalar(
                                out=out_t[:, hs], in0=ps_y[:, hs],
                                scalar1=bp_sb[:, ot:ot + 1],
                                scalar2=gate_sb[:, ot, b:b + 1],
                                op0=ALU.add, op1=ALU.mult)
                        nc.vector.tensor_add(out_t[:, hs], out_t[:, hs],
                                             x_sb[:, ot, b, hs])
                        nc.sync.dma_start(
                            out=out_d[b, ot * P:(ot + 1) * P, hs],
                            in_=out_t[:, hs])

            # ---- schedule ----
            emit_stats(0)
            emit_stats(1)
            emit_gn_pair(0)
            emit_se_pair(0)
            emit_qk(0)
            carry = None  # (b, xat) pending proj+fuse
            for b in range(BL):
                vt_sb = emit_vt(b)
                es_sb = emit_s(b)
                r_sb = emit_sums(es_sb)
                xat_sb = emit_av(b, vt_sb, es_sb, r_sb)
                if b == 0:
                    emit_stats(2)
                    emit_stats(3)
                if b == 1:
                    emit_gn_pair(1)
                    emit_se_pair(1)
                if b + 1 < BL:
                    emit_qk(b + 1)
                emit_proj_fuse(b, xat_sb, act_scale=(b == BL - 1))

    nc.compile()
    return nc


def _prep_inputs(x, gn_gamma, gn_beta, w_qkv, b_qkv, w_proj, b_proj,
                 w_se1, b_se1, w_se2, b_se2):
    bf = ml_dtypes.bfloat16
    f32 = np.float32

    def pt(w):  # [K, M] -> [128, K//128, M] partition-tiled
        K, M = w.shape
        return np.ascontiguousarray(w.reshape(K // P, P, M).transpose(1, 0, 2))

    wqk = pt(np.ascontiguousarray(w_qkv[:512].T)).astype(bf)       # [128,2,512]
    wv = pt(np.ascontiguousarray(w_qkv[512:].T)).astype(bf)        # [128,2,256]
    wp = pt(np.ascontiguousarray(w_proj.T)).astype(bf)             # [128,2,256]
    w1 = pt(np.ascontiguousarray(w_se1.T)).astype(f32)             # [128,2,64]
    w2 = np.ascontiguousarray(w_se2.T).astype(f32)                 # [64,256]

    def pcol(v):  # [256] -> [128, 2]
        return np.ascontiguousarray(v.reshape(2, P).T).astype(f32)

    gm = np.zeros((P, 16), f32)
    gm[np.arange(P), np.arange(P) // GSIZE] = 1.0
    shared = {
        "wqk": wqk, "wv": wv, "wp": wp, "w1": w1, "w2": w2,
        "gamma": pcol(gn_gamma), "beta": pcol(gn_beta),
        "bqk": np.ascontiguousarray(b_qkv[:512].reshape(4, P).T).astype(f32),
        "bv": pcol(b_qkv[512:]), "bp": pcol(b_proj),
        "b1": np.asarray(b_se1, f32).reshape(64, 1),
        "b2": pcol(b_se2),
        "gm": gm, "gmt": np.ascontiguousarray(gm.T),
        "ones": np.ones((P, P), bf),
    }
    xr = np.asarray(x, f32).reshape(B, C, N)
    in_maps = []
    for i in range(NCORES):
        m = dict(shared)
        m["x"] = np.ascontiguousarray(xr[i * BL:(i + 1) * BL])
        in_maps.append(m)
    want_bias_v = bool(np.any(np.asarray(b_qkv[512:]) != 0))
    return in_maps, want_bias_v


def _get_program(want_bias_v):
    key = ("prog", want_bias_v)
    if key not in _CACHE:
        _CACHE[key] = _build_program(want_bias_v)
    return _CACHE[key]


def run(inputs, trace=False, trace_kwargs=None):
    """Build + run on all 8 cores. Returns (full_out, BassKernelResults)."""
    from concourse.bass_utils import run_bass_kernel_spmd

    in_maps, want_bias_v = _prep_inputs(**inputs)
    nc = _get_program(want_bias_v)
    kw = {}
    if trace:
        kw["trace"] = True
        if trace_kwargs:
            kw["trace_kwargs"] = trace_kwargs
    res = run_bass_kernel_spmd(nc, in_maps, list(range(NCORES)), **kw)
    out = np.concatenate([res.results[i]["out"] for i in range(NCORES)], axis=0)
    return out.reshape(B, C, HW, HW).astype(np.float32), res


def kernel(**inputs):
    out, _ = run(inputs, trace=False)
    return out


# revision 16
# speedup vs baseline: 1.4714x; 1.0422x over previous
"""AttentionBlock (GroupNorm + single-head spatial attention + SE gate + residual)
Trainium2 Bass/Tile kernel, data-parallel over batch across 8 NeuronCores.

Full shapes: x [32, 256, 32, 32] f32 -> out [32, 256, 32, 32] f32.
Per core: 4 samples. Per sample (C=256, N=1024):
  xn = GroupNorm(x) (32 groups)            [C, N]  (bf16)
  q, k = Wqk @ xn                          [2C, N] (bf16, [c,n] layout)
  vT = xn^T @ WvT                          [N, C]  (bf16, [n,c] layout - direct!)
  esT = exp((k^T q) / 16)                  [N, N]  ([j, i] layout, j = softmax axis)
  sums_bc = ones128 @ esT  (accum over j)  [128, N] (each row = sum_j exp)
  r = 1/sums (reciprocal_approx_fast)      [128, N]
  xat = (vT^T @ esT) * r                   [C, N]  (unnormalized AV, scaled after)
  y = Wp @ xat                             [C, N]
  out = x + (y + bp) * gate[c]             (gate = SE sigmoid path from channel means)

No transposes anywhere: softmax reductions over j land on the PE contraction
axis (ones-matmul), the normalization is a rank-1 column scale folded in after
the AV matmul.
"""

import numpy as np
import ml_dtypes

B, C, HW, N = 32, 256, 32, 1024
NCORES = 8
BL = B // NCORES          # samples per core
GROUPS = 32
GSIZE = C // GROUPS       # 8 channels per group
EPS = 1e-5
CT = 2                    # channel partition tiles (256 = 2*128)
P = 128

_CACHE = {}


def _build_program(want_bias_v):
    import concourse.bacc as bacc
    import concourse.mybir as mybir
    import concourse.tile as tile

    f32 = mybir.dt.float32
    bf16 = mybir.dt.bfloat16
    AX = mybir.AxisListType.X
    AF = mybir.ActivationFunctionType
    ALU = mybir.AluOpType

    nc = bacc.Bacc()

    # ---- DRAM I/O ----
    x_d = nc.dram_tensor("x", [BL, C, N], f32, kind="ExternalInput")
    out_d = nc.dram_tensor("out", [BL, C, N], f32, kind="ExternalOutput")
    wqk_d = nc.dram_tensor("wqk", [P, 2, 512], bf16, kind="ExternalInput")
    wv_d = nc.dram_tensor("wv", [P, 2, C], bf16, kind="ExternalInput")
    wp_d = nc.dram_tensor("wp", [P, 2, C], bf16, kind="ExternalInput")
    w1_d = nc.dram_tensor("w1", [P, 2, 64], f32, kind="ExternalInput")
    w2_d = nc.dram_tensor("w2", [64, C], f32, kind="ExternalInput")
    gamma_d = nc.dram_tensor("gamma", [P, 2], f32, kind="ExternalInput")
    beta_d = nc.dram_tensor("beta", [P, 2], f32, kind="ExternalInput")
    bqk_d = nc.dram_tensor("bqk", [P, 4], f32, kind="ExternalInput")
    bv_d = nc.dram_tensor("bv", [P, 2], f32, kind="ExternalInput")
    bp_d = nc.dram_tensor("bp", [P, 2], f32, kind="ExternalInput")
    b1_d = nc.dram_tensor("b1", [64, 1], f32, kind="ExternalInput")
    b2_d = nc.dram_tensor("b2", [P, 2], f32, kind="ExternalInput")
    gm_d = nc.dram_tensor("gm", [P, 16], f32, kind="ExternalInput")
    gmt_d = nc.dram_tensor("gmt", [16, P], f32, kind="ExternalInput")
    ones_d = nc.dram_tensor("ones", [P, P], bf16, kind="ExternalInput")

    with tile.TileContext(nc) as tc:
        with (
            tc.tile_pool(name="persist", bufs=1) as persist,
            tc.tile_pool(name="qk", bufs=2) as qk_pool,
            tc.tile_pool(name="vt", bufs=2) as vt_pool,
            tc.tile_pool(name="es", bufs=2) as es_pool,
            tc.tile_pool(name="xat", bufs=2) as xat_pool,
            tc.tile_pool(name="rr", bufs=2) as r_pool,
            tc.tile_pool(name="junk", bufs=2) as junk_pool,
            tc.tile_pool(name="outp", bufs=3) as out_pool,
            tc.tile_pool(name="psb", bufs=3, space="PSUM") as psum_big,
            tc.tile_pool(name="pss", bufs=2, space="PSUM") as psum_small,
        ):
            # ---- DMA prologue: x slices first (sample 0's stats are the
            # critical path), then consts/weights in first-use order (the
            # HWDGE queue is FIFO in emission order).
            x_sb = persist.tile([P, CT, BL, N], f32)

            def load_x(b):
                for ct in range(CT):
                    nc.sync.dma_start(out=x_sb[:, ct, b],
                                      in_=x_d[b, ct * P:(ct + 1) * P, :])

            load_x(0)
            ones_sb = persist.tile([P, P], bf16)
            nc.sync.dma_start(out=ones_sb, in_=ones_d[:, :])
            load_x(1)
            gm_sb = persist.tile([P, 16], f32)
            nc.sync.dma_start(out=gm_sb, in_=gm_d[:, :])
            gmt_sb = persist.tile([16, P], f32)
            nc.sync.dma_start(out=gmt_sb, in_=gmt_d[:, :])
            gamma_sb = persist.tile([P, 2], f32)
            nc.sync.dma_start(out=gamma_sb, in_=gamma_d[:, :])
            beta_sb = persist.tile([P, 2], f32)
            nc.sync.dma_start(out=beta_sb, in_=beta_d[:, :])
            bqk_sb = persist.tile([P, 4], f32)
            nc.sync.dma_start(out=bqk_sb, in_=bqk_d[:, :])
            bv_sb = persist.tile([P, 2], f32)
            nc.sync.dma_start(out=bv_sb, in_=bv_d[:, :])
            bp_sb = persist.tile([P, 2], f32)
            nc.sync.dma_start(out=bp_sb, in_=bp_d[:, :])
            b1_sb = persist.tile([64, 1], f32)
            nc.sync.dma_start(out=b1_sb, in_=b1_d[:, :])
            b2_sb = persist.tile([P, 2], f32)
            nc.sync.dma_start(out=b2_sb, in_=b2_d[:, :])
            wqk_sb = persist.tile([P, 2, 512], bf16)
            nc.sync.dma_start(out=wqk_sb, in_=wqk_d[:, :, :])
            wv_sb = persist.tile([P, 2, C], bf16)
            nc.sync.dma_start(out=wv_sb, in_=wv_d[:, :, :])
            load_x(2)
            load_x(3)
            w1_sb = persist.tile([P, 2, 64], f32)
            nc.sync.dma_start(out=w1_sb, in_=w1_d[:, :, :])
            w2_sb = persist.tile([64, C], f32)
            nc.sync.dma_start(out=w2_sb, in_=w2_d[:, :])
            wp_sb = persist.tile([P, 2, C], bf16)
            nc.sync.dma_start(out=wp_sb, in_=wp_d[:, :, :])

            eps_sb = persist.tile([16, 1], f32)
            nc.vector.memset(eps_sb, EPS)
            nb2_sb = persist.tile([P, 2], f32)
            nc.vector.tensor_scalar_mul(nb2_sb, b2_sb, -1.0)

            # ---- persistent intermediates ----
            sums_c = persist.tile([P, CT, BL], f32)    # per-channel sums
            sumsq_c = persist.tile([P, CT, BL], f32)   # per-channel sum sq
            a_sb = persist.tile([P, CT, BL], f32)      # per-channel scale
            bb_sb = persist.tile([P, CT, BL], f32)     # per-channel offset
            xn_sb = persist.tile([P, CT, BL, N], bf16)
            gate_sb = persist.tile([P, CT, BL], f32)
            bpg_sb = persist.tile([P, CT, BL], f32)    # bp * gate (ACT fuse bias)
            h1_sb = persist.tile([64, BL], f32)
            qk_tiles = [None] * BL

            def emit_stats(b):
                for ct in range(CT):
                    nc.vector.reduce_sum(
                        out=sums_c[:, ct, b:b + 1], in_=x_sb[:, ct, b], axis=AX)
                    jt = junk_pool.tile([P, N], bf16, tag="junk")
                    nc.scalar.activation(
                        out=jt, in_=x_sb[:, ct, b], func=AF.Square,
                        accum_out=sumsq_c[:, ct, b:b + 1])

            def emit_gn_pair(p):
                pr = slice(2 * p, 2 * p + 2)
                for ct in range(CT):
                    ps_g = psum_small.tile([16, 4], f32, tag="pss")
                    nc.tensor.matmul(ps_g[:, 0:2], gm_sb, sums_c[:, ct, pr],
                                     start=True, stop=True)
                    nc.tensor.matmul(ps_g[:, 2:4], gm_sb, sumsq_c[:, ct, pr],
                                     start=True, stop=True)
                    nmean = persist.tile([16, 2], f32)
                    nc.vector.tensor_scalar_mul(nmean, ps_g[:, 0:2],
                                                -1.0 / (GSIZE * N))
                    var = persist.tile([16, 2], f32)
                    nc.vector.tensor_scalar_mul(var, ps_g[:, 2:4],
                                                1.0 / (GSIZE * N))
                    msq = persist.tile([16, 2], f32)
                    nc.vector.tensor_mul(msq, nmean, nmean)
                    nc.vector.tensor_sub(var, var, msq)
                    sd = persist.tile([16, 2], f32)
                    nc.scalar.activation(out=sd, in_=var, func=AF.Sqrt,
                                         bias=eps_sb)
                    rsm = persist.tile([16, 4], f32)
                    nc.vector.reciprocal(rsm[:, 0:2], sd)
                    nc.vector.tensor_mul(rsm[:, 2:4], nmean, rsm[:, 0:2])
                    ps_bc = psum_small.tile([P, 4], f32, tag="pss")
                    nc.tensor.matmul(ps_bc, gmt_sb, rsm, start=True, stop=True)
                    nc.vector.tensor_scalar_mul(a_sb[:, ct, pr], ps_bc[:, 0:2],
                                                gamma_sb[:, ct:ct + 1])
                    nc.vector.tensor_scalar(
                        out=bb_sb[:, ct, pr], in0=ps_bc[:, 2:4],
                        scalar1=gamma_sb[:, ct:ct + 1],
                        scalar2=beta_sb[:, ct:ct + 1],
                        op0=ALU.mult, op1=ALU.add)
                for bb in (2 * p, 2 * p + 1):
                    for ct in range(CT):
                        nc.gpsimd.tensor_scalar(
                            out=xn_sb[:, ct, bb], in0=x_sb[:, ct, bb],
                            scalar1=a_sb[:, ct, bb:bb + 1],
                            scalar2=bb_sb[:, ct, bb:bb + 1],
                            op0=ALU.mult, op1=ALU.add)

            def emit_se_pair(p):
                # sigmoid(z) = 1/(1+exp(-z)) so everything stays in the exp
                # activation-table set (no ACT table reloads mid-kernel)
                pr = slice(2 * p, 2 * p + 2)
                ps_h1 = psum_small.tile([64, 2], f32, tag="pss")
                for ct in range(CT):
                    nc.tensor.matmul(ps_h1, w1_sb[:, ct], sums_c[:, ct, pr],
                                     start=(ct == 0), stop=(ct == 1))
                nc.scalar.activation(out=h1_sb[:, pr], in_=ps_h1, func=AF.Relu,
                                     bias=b1_sb[:, 0:1], scale=1.0 / N)
                for ot in range(CT):
                    ps_gate = psum_small.tile([P, 2], f32, tag="pss")
                    nc.tensor.matmul(ps_gate, w2_sb[:, ot * P:(ot + 1) * P],
                                     h1_sb[:, pr], start=True, stop=True)
                    eg = persist.tile([P, 2], f32)
                    nc.scalar.activation(out=eg, in_=ps_gate, func=AF.Exp,
                                         scale=-1.0, bias=nb2_sb[:, ot:ot + 1])
                    nc.vector.tensor_scalar_add(eg, eg, 1.0)
                    nc.vector.reciprocal(gate_sb[:, ot, pr], eg)
                    nc.vector.tensor_scalar_mul(bpg_sb[:, ot, pr],
                                                gate_sb[:, ot, pr],
                                                bp_sb[:, ot:ot + 1])

            def emit_qk(b):
                # q, k : [c, n] layout. m-tile 0,1 = q rows; 2,3 = k rows.
                # Evac order q0,k0,q1,k1 so S's first K-step unblocks early.
                qk_sb = qk_pool.tile([P, 4, N], bf16, tag="qk")
                qk_tiles[b] = qk_sb
                for m in (0, 2, 1, 3):
                    ps_qk = psum_big.tile([P, N], f32, tag="psb")
                    for ns in range(2):
                        for kt in range(CT):
                            nc.tensor.matmul(
                                ps_qk[:, ns * 512:(ns + 1) * 512],
                                wqk_sb[:, kt, m * P:(m + 1) * P],
                                xn_sb[:, kt, b, ns * 512:(ns + 1) * 512],
                                start=(kt == 0), stop=(kt == 1))
                    nc.vector.tensor_scalar_add(qk_sb[:, m], ps_qk,
                                                bqk_sb[:, m:m + 1])

            def emit_vt(b):
                vt_sb = vt_pool.tile([P, 8, C], bf16, tag="vt")
                for jp in range(4):
                    ps_vt = psum_small.tile([P, 2, C], f32, tag="pss")
                    for j2 in range(2):
                        jt = 2 * jp + j2
                        for kt in range(CT):
                            nc.tensor.matmul(
                                ps_vt[:, j2],
                                xn_sb[:, kt, b, jt * P:(jt + 1) * P],
                                wv_sb[:, kt],
                                start=(kt == 0), stop=(kt == 1))
                    nc.scalar.activation(out=vt_sb[:, 2 * jp:2 * jp + 2],
                                         in_=ps_vt, func=AF.Copy)
                return vt_sb

            def emit_s(b):
                qk_sb = qk_tiles[b]
                es_sb = es_pool.tile([P, 8, N], bf16, tag="es")
                for mt in range(8):
                    ps_s = psum_big.tile([P, N], f32, tag="psb")
                    for ns in range(2):
                        for kt in range(CT):
                            nc.tensor.matmul(
                                ps_s[:, ns * 512:(ns + 1) * 512],
                                qk_sb[:, 2 + kt, mt * P:(mt + 1) * P],
                                qk_sb[:, kt, ns * 512:(ns + 1) * 512],
                                start=(kt == 0), stop=(kt == 1))
                    nc.scalar.activation(out=es_sb[:, mt], in_=ps_s,
                                         func=AF.Exp, scale=0.0625)
                return es_sb

            def emit_sums(es_sb):
                ps_sum = psum_big.tile([P, N], f32, tag="psb")
                for ns in range(2):
                    for jt in range(8):
                        nc.tensor.matmul(
                            ps_sum[:, ns * 512:(ns + 1) * 512],
                            ones_sb,
                            es_sb[:, jt, ns * 512:(ns + 1) * 512],
                            start=(jt == 0), stop=(jt == 7))
                r_sb = r_pool.tile([P, N], f32, tag="rr")
                nc.vector.reciprocal_approx_fast(out=r_sb, in_=ps_sum)
                return r_sb

            def emit_av(b, vt_sb, es_sb, r_sb):
                xat_sb = xat_pool.tile([P, CT, N], bf16, tag="xat")
                for ct2 in range(CT):
                    ps_av = psum_big.tile([P, N], f32, tag="psb")
                    for ns in range(2):
                        for jt in range(8):
                            nc.tensor.matmul(
                                ps_av[:, ns * 512:(ns + 1) * 512],
                                vt_sb[:, jt, ct2 * P:(ct2 + 1) * P],
                                es_sb[:, jt, ns * 512:(ns + 1) * 512],
                                start=(jt == 0), stop=(jt == 7))
                    if want_bias_v:
                        tmp = r_pool.tile([P, N], f32, tag="avtmp")
                        nc.vector.tensor_mul(tmp, ps_av, r_sb)
                        nc.vector.tensor_scalar_add(xat_sb[:, ct2], tmp,
                                                    bv_sb[:, ct2:ct2 + 1])
                    else:
                        nc.vector.tensor_mul(xat_sb[:, ct2], ps_av, r_sb)
                return xat_sb

            def emit_proj_fuse(b, xat_sb, act_scale):
                for ot in range(CT):
                    ps_y = psum_big.tile([P, N], f32, tag="psb")
                    for ns in range(2):
                        for kt2 in range(CT):
                            nc.tensor.matmul(
                                ps_y[:, ns * 512:(ns + 1) * 512],
                                wp_sb[:, kt2, ot * P:(ot + 1) * P],
                                xat_sb[:, kt2, ns * 512:(ns + 1) * 512],
                                start=(kt2 == 0), stop=(kt2 == 1))
                    out_t = out_pool.tile([P, N], f32, tag="outp")
                    for h in range(2):
                        hs = slice(h * 512, (h + 1) * 512)
                        if act_scale:
                            # (y + bp)*gate = y*gate + bp*gate on ScalarE,
                            # freeing VectorE for the residual adds at the tail
                            nc.scalar.activation(
                                out=out_t[:, hs], in_=ps_y[:, hs],
                                func=AF.Identity,
                                bias=bpg_sb[:, ot, b:b + 1],
                                scale=gate_sb[:, ot, b:b + 1])
                        else:
                            nc.vector.tensor_scalar(
                                out=out_t[:, hs], in0=ps_y[:, hs],
                                scalar1=bp_sb[:, ot:ot + 1],
                                scalar2=gate_sb[:, ot, b:b + 1],
                                op0=ALU.add, op1=ALU.mult)
                        nc.vector.tensor_add(out_t[:, hs], out_t[:, hs],
                                             x_sb[:, ot, b, hs])
                        nc.sync.dma_start(
                            out=out_d[b, ot * P:(ot + 1) * P, hs],
                            in_=out_t[:, hs])

            # ---- PE warm-up: dead matmuls during the DMA/stats head so
            # the HAM clock-gate reaches 8/8 before real matmuls ----
            warm_sb = persist.tile([P, 512], bf16)
            nc.vector.memset(warm_sb, 1.0)
            ps_warm = psum_big.tile([P, 512], f32, tag="psb")
            for _ in range(30):
                nc.tensor.matmul(ps_warm, warm_sb[:, 0:P], warm_sb,
                                 start=True, stop=True)

            # ---- schedule ----
            emit_stats(0)
            emit_stats(1)
            emit_gn_pair(0)
            emit_se_pair(0)
            emit_qk(0)
            carry = None  # (b, xat) pending proj+fuse
            for b in range(BL):
                vt_sb = emit_vt(b)
                es_sb = emit_s(b)
                r_sb = emit_sums(es_sb)
                xat_sb = emit_av(b, vt_sb, es_sb, r_sb)
                if b == 0:
                    emit_stats(2)
                    emit_stats(3)
                if b == 1:
                    emit_gn_pair(1)
                    emit_se_pair(1)
                if b + 1 < BL:
                    emit_qk(b + 1)
                emit_proj_fuse(b, xat_sb, act_scale=(b == BL - 1))

    nc.compile()
    return nc


def _prep_inputs(x, gn_gamma, gn_beta, w_qkv, b_qkv, w_proj, b_proj,
                 w_se1, b_se1, w_se2, b_se2):
    bf = ml_dtypes.bfloat16
    f32 = np.float32

    def pt(w):  # [K, M] -> [128, K//128, M] partition-tiled
        K, M = w.shape
        return np.ascontiguousarray(w.reshape(K // P, P, M).transpose(1, 0, 2))

    wqk = pt(np.ascontiguousarray(w_qkv[:512].T)).astype(bf)       # [128,2,512]
    wv = pt(np.ascontiguousarray(w_qkv[512:].T)).astype(bf)        # [128,2,256]
    wp = pt(np.ascontiguousarray(w_proj.T)).astype(bf)             # [128,2,256]
    w1 = pt(np.ascontiguousarray(w_se1.T)).astype(f32)             # [128,2,64]
    w2 = np.ascontiguousarray(w_se2.T).astype(f32)                 # [64,256]

    def pcol(v):  # [256] -> [128, 2]
        return np.ascontiguousarray(v.reshape(2, P).T).astype(f32)

    gm = np.zeros((P, 16), f32)
    gm[np.arange(P), np.arange(P) // GSIZE] = 1.0
    shared = {
        "wqk": wqk, "wv": wv, "wp": wp, "w1": w1, "w2": w2,
        "gamma": pcol(gn_gamma), "beta": pcol(gn_beta),
        "bqk": np.ascontiguousarray(b_qkv[:512].reshape(4, P).T).astype(f32),
        "bv": pcol(b_qkv[512:]), "bp": pcol(b_proj),
        "b1": np.asarray(b_se1, f32).reshape(64, 1),
        "b2": pcol(b_se2),
        "gm": gm, "gmt": np.ascontiguousarray(gm.T),
        "ones": np.ones((P, P), bf),
    }
    xr = np.asarray(x, f32).reshape(B, C, N)
    in_maps = []
    for i in range(NCORES):
        m = dict(shared)
        m["x"] = np.ascontiguousarray(xr[i * BL:(i + 1) * BL])
        in_maps.append(m)
    want_bias_v = bool(np.any(np.asarray(b_qkv[512:]) != 0))
    return in_maps, want_bias_v


def _get_program(want_bias_v):
    key = ("prog", want_bias_v)
    if key not in _CACHE:
        _CACHE[key] = _build_program(want_bias_v)
    return _CACHE[key]


def run(inputs, trace=False, trace_kwargs=None):
    """Build + run on all 8 cores. Returns (full_out, BassKernelResults)."""
    from concourse.bass_utils import run_bass_kernel_spmd

    in_maps, want_bias_v = _prep_inputs(**inputs)
    nc = _get_program(want_bias_v)
    kw = {}
    if trace:
        kw["trace"] = True
        if trace_kwargs:
            kw["trace_kwargs"] = trace_kwargs
    res = run_bass_kernel_spmd(nc, in_maps, list(range(NCORES)), **kw)
    out = np.concatenate([res.results[i]["out"] for i in range(NCORES)], axis=0)
    return out.reshape(B, C, HW, HW).astype(np.float32), res


def kernel(**inputs):
    out, _ = run(inputs, trace=False)
    return out


# revision 18
# speedup vs baseline: 1.5209x; 1.0336x over previous
"""AttentionBlock (GroupNorm + single-head spatial attention + SE gate + residual)
Trainium2 Bass/Tile kernel, data-parallel over batch across 8 NeuronCores.

Full shapes: x [32, 256, 32, 32] f32 -> out [32, 256, 32, 32] f32.
Per core: 4 samples. Per sample (C=256, N=1024):
  xn = GroupNorm(x) (32 groups)            [C, N]  (bf16)
  q, k = Wqk @ xn                          [2C, N] (bf16, [c,n] layout)
  vT = xn^T @ WvT                          [N, C]  (bf16, [n,c] layout - direct!)
  esT = exp((k^T q) / 16)                  [N, N]  ([j, i] layout, j = softmax axis)
  sums_bc = ones128 @ esT  (accum over j)  [128, N] (each row = sum_j exp)
  r = 1/sums (reciprocal_approx_fast)      [128, N]
  xat = (vT^T @ esT) * r                   [C, N]  (unnormalized AV, scaled after)
  y = Wp @ xat                             [C, N]
  out = x + (y + bp) * gate[c]             (gate = SE sigmoid path from channel means)

No transposes anywhere: softmax reductions over j land on the PE contraction
axis (ones-matmul), the normalization is a rank-1 column scale folded in after
the AV matmul.
"""

import numpy as np
import ml_dtypes

B, C, HW, N = 32, 256, 32, 1024
NCORES = 8
BL = B // NCORES          # samples per core
GROUPS = 32
GSIZE = C // GROUPS       # 8 channels per group
EPS = 1e-5
CT = 2                    # channel partition tiles (256 = 2*128)
P = 128

_CACHE = {}


def _build_program(want_bias_v):
    import concourse.bacc as bacc
    import concourse.mybir as mybir
    import concourse.tile as tile

    f32 = mybir.dt.float32
    bf16 = mybir.dt.bfloat16
    AX = mybir.AxisListType.X
    AF = mybir.ActivationFunctionType
    ALU = mybir.AluOpType

    nc = bacc.Bacc()

    # ---- DRAM I/O ----
    x_d = nc.dram_tensor("x", [BL, C, N], f32, kind="ExternalInput")
    out_d = nc.dram_tensor("out", [BL, C, N], f32, kind="ExternalOutput")
    wqk_d = nc.dram_tensor("wqk", [P, 2, 512], bf16, kind="ExternalInput")
    wv_d = nc.dram_tensor("wv", [P, 2, C], bf16, kind="ExternalInput")
    wp_d = nc.dram_tensor("wp", [P, 2, C], bf16, kind="ExternalInput")
    w1_d = nc.dram_tensor("w1", [P, 2, 64], f32, kind="ExternalInput")
    w2_d = nc.dram_tensor("w2", [64, C], f32, kind="ExternalInput")
    gamma_d = nc.dram_tensor("gamma", [P, 2], f32, kind="ExternalInput")
    beta_d = nc.dram_tensor("beta", [P, 2], f32, kind="ExternalInput")
    bqk_d = nc.dram_tensor("bqk", [P, 4], f32, kind="ExternalInput")
    bv_d = nc.dram_tensor("bv", [P, 2], f32, kind="ExternalInput")
    bp_d = nc.dram_tensor("bp", [P, 2], f32, kind="ExternalInput")
    b1_d = nc.dram_tensor("b1", [64, 1], f32, kind="ExternalInput")
    b2_d = nc.dram_tensor("b2", [P, 2], f32, kind="ExternalInput")
    gm_d = nc.dram_tensor("gm", [P, 16], f32, kind="ExternalInput")
    gmt_d = nc.dram_tensor("gmt", [16, P], f32, kind="ExternalInput")
    ones_d = nc.dram_tensor("ones", [P, P], bf16, kind="ExternalInput")

    with tile.TileContext(nc) as tc:
        with (
            tc.tile_pool(name="persist", bufs=1) as persist,
            tc.tile_pool(name="qk", bufs=2) as qk_pool,
            tc.tile_pool(name="vt", bufs=2) as vt_pool,
            tc.tile_pool(name="es", bufs=2) as es_pool,
            tc.tile_pool(name="xat", bufs=2) as xat_pool,
            tc.tile_pool(name="rr", bufs=2) as r_pool,
            tc.tile_pool(name="junk", bufs=2) as junk_pool,
            tc.tile_pool(name="outp", bufs=3) as out_pool,
            tc.tile_pool(name="psb", bufs=3, space="PSUM") as psum_big,
            tc.tile_pool(name="pss", bufs=2, space="PSUM") as psum_small,
        ):
            # ---- DMA prologue: x slices first (sample 0's stats are the
            # critical path), then consts/weights in first-use order (the
            # HWDGE queue is FIFO in emission order).
            x_sb = persist.tile([P, CT, BL, N], f32)

            def load_x(b):
                for ct in range(CT):
                    nc.sync.dma_start(out=x_sb[:, ct, b],
                                      in_=x_d[b, ct * P:(ct + 1) * P, :])

            load_x(0)
            ones_sb = persist.tile([P, P], bf16)
            nc.sync.dma_start(out=ones_sb, in_=ones_d[:, :])
            load_x(1)
            gm_sb = persist.tile([P, 16], f32)
            nc.sync.dma_start(out=gm_sb, in_=gm_d[:, :])
            gmt_sb = persist.tile([16, P], f32)
            nc.sync.dma_start(out=gmt_sb, in_=gmt_d[:, :])
            gamma_sb = persist.tile([P, 2], f32)
            nc.sync.dma_start(out=gamma_sb, in_=gamma_d[:, :])
            beta_sb = persist.tile([P, 2], f32)
            nc.sync.dma_start(out=beta_sb, in_=beta_d[:, :])
            bqk_sb = persist.tile([P, 4], f32)
            nc.sync.dma_start(out=bqk_sb, in_=bqk_d[:, :])
            bv_sb = persist.tile([P, 2], f32)
            nc.sync.dma_start(out=bv_sb, in_=bv_d[:, :])
            bp_sb = persist.tile([P, 2], f32)
            nc.sync.dma_start(out=bp_sb, in_=bp_d[:, :])
            b1_sb = persist.tile([64, 1], f32)
            nc.sync.dma_start(out=b1_sb, in_=b1_d[:, :])
            b2_sb = persist.tile([P, 2], f32)
            nc.sync.dma_start(out=b2_sb, in_=b2_d[:, :])
            wqk_sb = persist.tile([P, 2, 512], bf16)
            nc.sync.dma_start(out=wqk_sb, in_=wqk_d[:, :, :])
            wv_sb = persist.tile([P, 2, C], bf16)
            nc.sync.dma_start(out=wv_sb, in_=wv_d[:, :, :])
            load_x(2)
            load_x(3)
            w1_sb = persist.tile([P, 2, 64], f32)
            nc.sync.dma_start(out=w1_sb, in_=w1_d[:, :, :])
            w2_sb = persist.tile([64, C], f32)
            nc.sync.dma_start(out=w2_sb, in_=w2_d[:, :])
            wp_sb = persist.tile([P, 2, C], bf16)
            nc.sync.dma_start(out=wp_sb, in_=wp_d[:, :, :])

            eps_sb = persist.tile([16, 1], f32)
            nc.vector.memset(eps_sb, EPS)
            nb2_sb = persist.tile([P, 2], f32)
            nc.vector.tensor_scalar_mul(nb2_sb, b2_sb, -1.0)

            # ---- persistent intermediates ----
            sums_c = persist.tile([P, CT, BL], f32)    # per-channel sums
            sumsq_c = persist.tile([P, CT, BL], f32)   # per-channel sum sq
            a_sb = persist.tile([P, CT, BL], f32)      # per-channel scale
            bb_sb = persist.tile([P, CT, BL], f32)     # per-channel offset
            xn_sb = persist.tile([P, CT, BL, N], bf16)
            gate_sb = persist.tile([P, CT, BL], f32)
            bpg_sb = persist.tile([P, CT, BL], f32)    # bp * gate (ACT fuse bias)
            h1_sb = persist.tile([64, BL], f32)
            qk_tiles = [None] * BL

            def emit_stats(b):
                for ct in range(CT):
                    nc.vector.reduce_sum(
                        out=sums_c[:, ct, b:b + 1], in_=x_sb[:, ct, b], axis=AX)
                    jt = junk_pool.tile([P, N], bf16, tag="junk")
                    nc.scalar.activation(
                        out=jt, in_=x_sb[:, ct, b], func=AF.Square,
                        accum_out=sumsq_c[:, ct, b:b + 1])

            def emit_gn_pair(p):
                pr = slice(2 * p, 2 * p + 2)
                for ct in range(CT):
                    ps_g = psum_small.tile([16, 4], f32, tag="pss")
                    nc.tensor.matmul(ps_g[:, 0:2], gm_sb, sums_c[:, ct, pr],
                                     start=True, stop=True)
                    nc.tensor.matmul(ps_g[:, 2:4], gm_sb, sumsq_c[:, ct, pr],
                                     start=True, stop=True)
                    nmean = persist.tile([16, 2], f32)
                    nc.vector.tensor_scalar_mul(nmean, ps_g[:, 0:2],
                                                -1.0 / (GSIZE * N))
                    var = persist.tile([16, 2], f32)
                    nc.vector.tensor_scalar_mul(var, ps_g[:, 2:4],
                                                1.0 / (GSIZE * N))
                    msq = persist.tile([16, 2], f32)
                    nc.vector.tensor_mul(msq, nmean, nmean)
                    nc.vector.tensor_sub(var, var, msq)
                    sd = persist.tile([16, 2], f32)
                    nc.scalar.activation(out=sd, in_=var, func=AF.Sqrt,
                                         bias=eps_sb)
                    rsm = persist.tile([16, 4], f32)
                    nc.vector.reciprocal(rsm[:, 0:2], sd)
                    nc.vector.tensor_mul(rsm[:, 2:4], nmean, rsm[:, 0:2])
                    ps_bc = psum_small.tile([P, 4], f32, tag="pss")
                    nc.tensor.matmul(ps_bc, gmt_sb, rsm, start=True, stop=True)
                    nc.vector.tensor_scalar_mul(a_sb[:, ct, pr], ps_bc[:, 0:2],
                                                gamma_sb[:, ct:ct + 1])
                    nc.vector.tensor_scalar(
                        out=bb_sb[:, ct, pr], in0=ps_bc[:, 2:4],
                        scalar1=gamma_sb[:, ct:ct + 1],
                        scalar2=beta_sb[:, ct:ct + 1],
                        op0=ALU.mult, op1=ALU.add)
                for bb in (2 * p, 2 * p + 1):
                    for ct in range(CT):
                        nc.gpsimd.tensor_scalar(
                            out=xn_sb[:, ct, bb], in0=x_sb[:, ct, bb],
                            scalar1=a_sb[:, ct, bb:bb + 1],
                            scalar2=bb_sb[:, ct, bb:bb + 1],
                            op0=ALU.mult, op1=ALU.add)

            def emit_se_pair(p):
                # sigmoid(z) = 1/(1+exp(-z)) so everything stays in the exp
                # activation-table set (no ACT table reloads mid-kernel)
                pr = slice(2 * p, 2 * p + 2)
                ps_h1 = psum_small.tile([64, 2], f32, tag="pss")
                for ct in range(CT):
                    nc.tensor.matmul(ps_h1, w1_sb[:, ct], sums_c[:, ct, pr],
                                     start=(ct == 0), stop=(ct == 1))
                nc.scalar.activation(out=h1_sb[:, pr], in_=ps_h1, func=AF.Relu,
                                     bias=b1_sb[:, 0:1], scale=1.0 / N)
                for ot in range(CT):
                    ps_gate = psum_small.tile([P, 2], f32, tag="pss")
                    nc.tensor.matmul(ps_gate, w2_sb[:, ot * P:(ot + 1) * P],
                                     h1_sb[:, pr], start=True, stop=True)
                    eg = persist.tile([P, 2], f32)
                    nc.scalar.activation(out=eg, in_=ps_gate, func=AF.Exp,
                                         scale=-1.0, bias=nb2_sb[:, ot:ot + 1])
                    nc.vector.tensor_scalar_add(eg, eg, 1.0)
                    nc.vector.reciprocal(gate_sb[:, ot, pr], eg)
                    nc.vector.tensor_scalar_mul(bpg_sb[:, ot, pr],
                                                gate_sb[:, ot, pr],
                                                bp_sb[:, ot:ot + 1])

            def emit_qk(b):
                # q, k : [c, n] layout. m-tile 0,1 = q rows; 2,3 = k rows.
                # Evac order q0,k0,q1,k1 so S's first K-step unblocks early.
                qk_sb = qk_pool.tile([P, 4, N], bf16, tag="qk")
                qk_tiles[b] = qk_sb
                for m in (0, 2, 1, 3):
                    ps_qk = psum_big.tile([P, N], f32, tag="psb")
                    for ns in range(2):
                        for kt in range(CT):
                            nc.tensor.matmul(
                                ps_qk[:, ns * 512:(ns + 1) * 512],
                                wqk_sb[:, kt, m * P:(m + 1) * P],
                                xn_sb[:, kt, b, ns * 512:(ns + 1) * 512],
                                start=(kt == 0), stop=(kt == 1))
                    if m >= 2:  # k evac on ScalarE, q on VectorE: parallel
                        nc.scalar.activation(out=qk_sb[:, m], in_=ps_qk,
                                             func=AF.Identity,
                                             bias=bqk_sb[:, m:m + 1])
                    else:
                        nc.vector.tensor_scalar_add(qk_sb[:, m], ps_qk,
                                                    bqk_sb[:, m:m + 1])

            def emit_vt(b):
                vt_sb = vt_pool.tile([P, 8, C], bf16, tag="vt")
                for jp in range(4):
                    ps_vt = psum_small.tile([P, 2, C], f32, tag="pss")
                    for j2 in range(2):
                        jt = 2 * jp + j2
                        for kt in range(CT):
                            nc.tensor.matmul(
                                ps_vt[:, j2],
                                xn_sb[:, kt, b, jt * P:(jt + 1) * P],
                                wv_sb[:, kt],
                                start=(kt == 0), stop=(kt == 1))
                    nc.scalar.activation(out=vt_sb[:, 2 * jp:2 * jp + 2],
                                         in_=ps_vt, func=AF.Copy)
                return vt_sb

            def emit_s(b):
                qk_sb = qk_tiles[b]
                es_sb = es_pool.tile([P, 8, N], bf16, tag="es")
                for mt in range(8):
                    ps_s = psum_big.tile([P, N], f32, tag="psb")
                    for ns in range(2):
                        for kt in range(CT):
                            nc.tensor.matmul(
                                ps_s[:, ns * 512:(ns + 1) * 512],
                                qk_sb[:, 2 + kt, mt * P:(mt + 1) * P],
                                qk_sb[:, kt, ns * 512:(ns + 1) * 512],
                                start=(kt == 0), stop=(kt == 1))
                    nc.scalar.activation(out=es_sb[:, mt], in_=ps_s,
                                         func=AF.Exp, scale=0.0625)
                return es_sb

            def emit_sums(es_sb):
                ps_sum = psum_big.tile([P, N], f32, tag="psb")
                for ns in range(2):
                    for jt in range(8):
                        nc.tensor.matmul(
                            ps_sum[:, ns * 512:(ns + 1) * 512],
                            ones_sb,
                            es_sb[:, jt, ns * 512:(ns + 1) * 512],
                            start=(jt == 0), stop=(jt == 7))
                r_sb = r_pool.tile([P, N], f32, tag="rr")
                nc.vector.reciprocal_approx_fast(out=r_sb, in_=ps_sum)
                return r_sb

            def emit_av(b, vt_sb, es_sb, r_sb):
                xat_sb = xat_pool.tile([P, CT, N], bf16, tag="xat")
                ps_avs = [psum_big.tile([P, N], f32, tag="psb",
                                        name=f"ps_av{_i}") for _i in range(CT)]
                for ns in range(2):
                    hs = slice(ns * 512, (ns + 1) * 512)
                    for ct2 in range(CT):
                        for jt in range(8):
                            nc.tensor.matmul(
                                ps_avs[ct2][:, hs],
                                vt_sb[:, jt, ct2 * P:(ct2 + 1) * P],
                                es_sb[:, jt, hs],
                                start=(jt == 0), stop=(jt == 7))
                        if want_bias_v:
                            tmp = r_pool.tile([P, 512], f32, tag="avtmp")
                            nc.vector.tensor_mul(tmp, ps_avs[ct2][:, hs],
                                                 r_sb[:, hs])
                            nc.vector.tensor_scalar_add(
                                xat_sb[:, ct2, hs], tmp,
                                bv_sb[:, ct2:ct2 + 1])
                        else:
                            nc.vector.tensor_mul(xat_sb[:, ct2, hs],
                                                 ps_avs[ct2][:, hs],
                                                 r_sb[:, hs])
                return xat_sb

            def emit_proj_fuse(b, xat_sb, act_scale):
                ps_ys = [psum_big.tile([P, N], f32, tag="psb",
                                       name=f"ps_y{_i}") for _i in range(CT)]
                out_ts = [out_pool.tile([P, N], f32, tag="outp",
                                        name=f"out_t{_i}") for _i in range(CT)]
                for h in range(2):
                    hs = slice(h * 512, (h + 1) * 512)
                    for ot in range(CT):
                        for kt2 in range(CT):
                            nc.tensor.matmul(
                                ps_ys[ot][:, hs],
                                wp_sb[:, kt2, ot * P:(ot + 1) * P],
                                xat_sb[:, kt2, hs],
                                start=(kt2 == 0), stop=(kt2 == 1))
                        out_t = out_ts[ot]
                        if act_scale:
                            # (y + bp)*gate = y*gate + bp*gate on ScalarE,
                            # freeing VectorE for the residual adds at the tail
                            nc.scalar.activation(
                                out=out_t[:, hs], in_=ps_ys[ot][:, hs],
                                func=AF.Identity,
                                bias=bpg_sb[:, ot, b:b + 1],
                                scale=gate_sb[:, ot, b:b + 1])
                        else:
                            nc.vector.tensor_scalar(
                                out=out_t[:, hs], in0=ps_ys[ot][:, hs],
                                scalar1=bp_sb[:, ot:ot + 1],
                                scalar2=gate_sb[:, ot, b:b + 1],
                                op0=ALU.add, op1=ALU.mult)
                        nc.vector.tensor_add(out_t[:, hs], out_t[:, hs],
                                             x_sb[:, ot, b, hs])
                        nc.sync.dma_start(
                            out=out_d[b, ot * P:(ot + 1) * P, hs],
                            in_=out_t[:, hs])

            # ---- PE warm-up: dead matmuls during the DMA/stats head so
            # the HAM clock-gate reaches 8/8 before real matmuls ----
            warm_sb = persist.tile([P, 512], bf16)
            nc.vector.memset(warm_sb, 1.0)
            ps_warm = psum_big.tile([P, 512], f32, tag="psb")
            for _ in range(30):
                nc.tensor.matmul(ps_warm, warm_sb[:, 0:P], warm_sb,
                                 start=True, stop=True)

            # ---- schedule ----
            emit_stats(0)
            emit_stats(1)
            emit_gn_pair(0)
            emit_se_pair(0)
            emit_qk(0)
            carry = None  # (b, xat) pending proj+fuse
            for b in range(BL):
                vt_sb = emit_vt(b)
                es_sb = emit_s(b)
                r_sb = emit_sums(es_sb)
                xat_sb = emit_av(b, vt_sb, es_sb, r_sb)
                if b + 1 < BL:
                    emit_qk(b + 1)
                if b == 0:
                    emit_stats(2)
                    emit_stats(3)
                    emit_gn_pair(1)
                    emit_se_pair(1)
                emit_proj_fuse(b, xat_sb, act_scale=(b == BL - 1))

    nc.compile()
    return nc


def _prep_inputs(x, gn_gamma, gn_beta, w_qkv, b_qkv, w_proj, b_proj,
                 w_se1, b_se1, w_se2, b_se2):
    bf = ml_dtypes.bfloat16
    f32 = np.float32

    def pt(w):  # [K, M] -> [128, K//128, M] partition-tiled
        K, M = w.shape
        return np.ascontiguousarray(w.reshape(K // P, P, M).transpose(1, 0, 2))

    wqk = pt(np.ascontiguousarray(w_qkv[:512].T)).astype(bf)       # [128,2,512]
    wv = pt(np.ascontiguousarray(w_qkv[512:].T)).astype(bf)        # [128,2,256]
    wp = pt(np.ascontiguousarray(w_proj.T)).astype(bf)             # [128,2,256]
    w1 = pt(np.ascontiguousarray(w_se1.T)).astype(f32)             # [128,2,64]
    w2 = np.ascontiguousarray(w_se2.T).astype(f32)                 # [64,256]

    def pcol(v):  # [256] -> [128, 2]
        return np.ascontiguousarray(v.reshape(2, P).T).astype(f32)

    gm = np.zeros((P, 16), f32)
    gm[np.arange(P), np.arange(P) // GSIZE] = 1.0
    shared = {
        "wqk": wqk, "wv": wv, "wp": wp, "w1": w1, "w2": w2,
        "gamma": pcol(gn_gamma), "beta": pcol(gn_beta),
        "bqk": np.ascontiguousarray(b_qkv[:512].reshape(4, P).T).astype(f32),
        "bv": pcol(b_qkv[512:]), "bp": pcol(b_proj),
        "b1": np.asarray(b_se1, f32).reshape(64, 1),
        "b2": pcol(b_se2),
        "gm": gm, "gmt": np.ascontiguousarray(gm.T),
        "ones": np.ones((P, P), bf),
    }
    xr = np.asarray(x, f32).reshape(B, C, N)
    in_maps = []
    for i in range(NCORES):
        m = dict(shared)
        m["x"] = np.ascontiguousarray(xr[i * BL:(i + 1) * BL])
        in_maps.append(m)
    want_bias_v = bool(np.any(np.asarray(b_qkv[512:]) != 0))
    return in_maps, want_bias_v


def _get_program(want_bias_v):
    key = ("prog", want_bias_v)
    if key not in _CACHE:
        _CACHE[key] = _build_program(want_bias_v)
    return _CACHE[key]


def run(inputs, trace=False, trace_kwargs=None):
    """Build + run on all 8 cores. Returns (full_out, BassKernelResults)."""
    from concourse.bass_utils import run_bass_kernel_spmd

    in_maps, want_bias_v = _prep_inputs(**inputs)
    nc = _get_program(want_bias_v)
    kw = {}
    if trace:
        kw["trace"] = True
        if trace_kwargs:
            kw["trace_kwargs"] = trace_kwargs
    res = run_bass_kernel_spmd(nc, in_maps, list(range(NCORES)), **kw)
    out = np.concatenate([res.results[i]["out"] for i in range(NCORES)], axis=0)
    return out.reshape(B, C, HW, HW).astype(np.float32), res


def kernel(**inputs):
    out, _ = run(inputs, trace=False)
    return out
